# revision 25
# baseline (speedup 1.0000x reference)
"""MoE fusion kernel for Trainium2 (8 NeuronCores, two-phase sparse routing).

Phase 1 (gate NEFF, all-bf16): computes gate logits on device.  bf16 logit
error is <= ~1e-2; any token whose 2nd-vs-3rd logit margin is below
MARGIN_THR is recomputed exactly (fp32) on the host before routing, so the
top-2 selection always matches the fp32 reference.  The combine weights
collapse to sigmoid(l1 - l2), which tolerates ~1e-2 logit error harmlessly.

Phase 2 (expert NEFF, bf16): host routes each token's top-2 experts into
24 uniform slots (3 per core, capacity S).  Each slot runs one expert's
Linear-GELU-Linear over its gathered tokens in a feature-major layout:

    x.T [1536, S] -> h.T = gelu(W1.T x.T) [3072, S] -> o.T = W2.T h.T [768, S]

Matmuls are weights-stationary with 512-column moving chunks (PSUM bank
limit); each matmul instruction costs ~N/2.4 + 43 ns, so big-N chunks and
low padding dominate the schedule.  Device returns sigmoid(o) in bf16; the
host applies the combine weights during scatter-add.
"""

import numpy as np

try:
    import concourse  # noqa: F401
except ImportError:  # pragma: no cover
    import sys

    sys.path.insert(0, "/opt/trn_rl_repo")

import concourse.bass as bass  # noqa: F401
import concourse.mybir as mybir
import concourse.tile as tile
from concourse import bacc
from concourse.bass_utils import run_bass_kernel_spmd

# Problem shapes (hardcoded per contest rules).
N, D, E, H, NE = 8192, 1536, 768, 3072, 12
NCORES = 8
T = N // NCORES  # 1024 tokens per core
P = 128
KO1 = D // P  # 12   k-tiles of the first expert matmul
FO1 = H // P  # 24   feature-tiles of h
KO2 = H // P  # 24   k-tiles of the second expert matmul
FO2 = E // P  # 6    feature-tiles of the output
GFO = E // P  # 6    feature-tiles of the gate hidden

F32 = mybir.dt.float32
BF16 = mybir.dt.bfloat16
AF = mybir.ActivationFunctionType
OP = mybir.AluOpType

GELU = AF.Gelu  # test.py sim-mode substitutes Tanh (CoreSim lacks Gelu)

EXP = 5  # expert slots per core (8*EXP slots total, assigned by load)
SUMS_MAX = 4096  # cap on per-core total slot capacity (SBUF) -> dense fallback
MARGIN_THR = 0.03  # host-recompute tokens with 2nd-3rd logit margin below this


def _chunks(total):
    """Column chunks <=512 (PSUM bank limit); even split for 512<S<=1024 so
    neither chunk drops under the ~100ns LDWEIGHTS issue floor."""
    if total <= 512:
        return [(0, total)]
    assert total <= 1024
    h = (total + 1) // 2
    return [(0, h), (h, total)]


# ======================================================================
# Phase-1 NEFF: bf16 gate -> logits.T [NE, T] per core
# ======================================================================


def build_nc_gate():
    nc = bacc.Bacc("TRN2", target_bir_lowering=False, debug=False, num_devices=NCORES)
    xTb = nc.dram_tensor("xTb", [P, KO1, T], BF16, kind="ExternalInput").ap()
    gw1 = nc.dram_tensor("gw1", [P, GFO, KO1, P], BF16, kind="ExternalInput").ap()
    gb1 = nc.dram_tensor("gb1", [P, GFO], F32, kind="ExternalInput").ap()
    gw2 = nc.dram_tensor("gw2", [P, GFO, NE], BF16, kind="ExternalInput").ap()
    lgT = nc.dram_tensor("lgT", [NE, T], F32, kind="ExternalOutput").ap()

    with tile.TileContext(nc) as tc:
        with (
            tc.tile_pool(name="sb", bufs=1) as sb,
            tc.tile_pool(name="lg", bufs=2) as lg,
            tc.tile_pool(name="ps", bufs=2, space="PSUM") as ps,
            tc.tile_pool(name="pl", bufs=2, space="PSUM") as pls,
        ):
            gw1_s = sb.tile([P, GFO, KO1, P], BF16)
            xTb_s = sb.tile([P, KO1, T], BF16)
            gb1_s = sb.tile([P, GFO], F32)
            gw2_s = sb.tile([P, GFO, NE], BF16)
            # interleave so that (gw1[fo0], xTb[t2=0]) land first and gw1[fo1]
            # is not queued behind the whole first xTb chunk
            nc.sync.dma_start(gw1_s[:, 0], gw1[:, 0])
            nc.sync.dma_start(xTb_s[:, : KO1 // 2, 0:512], xTb[:, : KO1 // 2, 0:512])
            nc.sync.dma_start(gw1_s[:, 1], gw1[:, 1])
            nc.sync.dma_start(xTb_s[:, KO1 // 2 :, 0:512], xTb[:, KO1 // 2 :, 0:512])
            for fo in range(2, GFO):
                nc.sync.dma_start(gw1_s[:, fo], gw1[:, fo])
            nc.sync.dma_start(gb1_s[:], gb1)
            nc.sync.dma_start(gw2_s[:], gw2)
            for t2 in range(1, T // 512):
                nc.sync.dma_start(
                    xTb_s[:, :, t2 * 512 : (t2 + 1) * 512],
                    xTb[:, :, t2 * 512 : (t2 + 1) * 512],
                )
            ghT = sb.tile([P, GFO, T], BF16)

            for t2 in range(T // 512):
                sl = slice(t2 * 512, (t2 + 1) * 512)
                for fo in range(GFO):
                    pg = ps.tile([P, 512], F32, tag="pg")
                    for ko in range(KO1):
                        nc.tensor.matmul(
                            pg[:],
                            lhsT=gw1_s[:, fo, ko, :],
                            rhs=xTb_s[:, ko, sl],
                            start=(ko == 0),
                            stop=(ko == KO1 - 1),
                        )
                    nc.scalar.activation(
                        ghT[:, fo, sl], pg[:], GELU, bias=gb1_s[:, fo : fo + 1]
                    )
                pl = pls.tile([NE, 512], F32, tag="pl")
                for fo in range(GFO):
                    nc.tensor.matmul(
                        pl[:],
                        lhsT=gw2_s[:, fo, :],
                        rhs=ghT[:, fo, sl],
                        start=(fo == 0),
                        stop=(fo == GFO - 1),
                    )
                lt = lg.tile([NE, 512], F32, tag="lt")
                nc.vector.tensor_copy(lt[:], pl[:])
                nc.sync.dma_start(lgT[:, sl], lt[:])
    nc.compile()
    return nc


# ======================================================================
# Phase-2 NEFF: per-core EXP expert slots with per-slot capacities `sizes`
# (identical across cores; slot loads are classed by the host router).
# ======================================================================


def build_nc_exp(sizes):
    sizes = tuple(int(s) for s in sizes)
    Tc = sum(sizes)
    offs = [0]
    for s in sizes:
        offs.append(offs[-1] + s)
    nc = bacc.Bacc("TRN2", target_bir_lowering=False, debug=False, num_devices=NCORES)
    xTe = nc.dram_tensor("xTe", [P, KO1, Tc], BF16, kind="ExternalInput").ap()
    w1s = nc.dram_tensor("w1s", [EXP, FO1, P, KO1, P], BF16, kind="ExternalInput").ap()
    b1s = nc.dram_tensor("b1s", [P, EXP, FO1], F32, kind="ExternalInput").ap()
    w2s = nc.dram_tensor("w2s", [EXP, FO2, P, KO2, P], BF16, kind="ExternalInput").ap()
    b2s = nc.dram_tensor("b2s", [P, EXP, FO2], F32, kind="ExternalInput").ap()
    oT = nc.dram_tensor("oT", [P, FO2, Tc], BF16, kind="ExternalOutput").ap()

    import contextlib

    with tile.TileContext(nc) as tc, contextlib.ExitStack() as ctx:
        pers = ctx.enter_context(tc.tile_pool(name="pers", bufs=1))
        b1s_s = pers.tile([P, EXP, FO1], F32)
        b2s_s = pers.tile([P, EXP, FO2], F32)
        xTe_s = pers.tile([P, KO1, Tc], BF16)

        def fetch_xte(sl):
            # one strided DMA for the whole slot (DMA-issue on Sync costs
            # ~0.65us each; piecewise issue serializes the startup)
            nc.sync.dma_start(
                xTe_s[:, :, offs[sl] : offs[sl + 1]],
                xTe[:, :, offs[sl] : offs[sl + 1]],
            )

        # slot 0's tokens land first (two ko-halves so the fo0 accumulation
        # can start after half the transfer); later slots prefetch below
        nc.sync.dma_start(
            xTe_s[:, : KO1 // 2, offs[0] : offs[1]],
            xTe[:, : KO1 // 2, offs[0] : offs[1]],
        )
        nc.sync.dma_start(
            xTe_s[:, KO1 // 2 :, offs[0] : offs[1]],
            xTe[:, KO1 // 2 :, offs[0] : offs[1]],
        )
        nc.sync.dma_start(b1s_s[:], b1s)  # biases after: consumed only by ACT
        nc.sync.dma_start(b2s_s[:], b2s)

        w1pool = ctx.enter_context(tc.tile_pool(name="w1p", bufs=6))
        w2pool = ctx.enter_context(tc.tile_pool(name="w2p", bufs=4))
        hpool = ctx.enter_context(tc.tile_pool(name="hp", bufs=1))
        spool = ctx.enter_context(tc.tile_pool(name="sp", bufs=4))
        opool = ctx.enter_context(tc.tile_pool(name="op", bufs=4))
        psA = ctx.enter_context(tc.tile_pool(name="psA", bufs=4, space="PSUM"))
        psB = ctx.enter_context(tc.tile_pool(name="psB", bufs=4, space="PSUM"))

        for sl in range(EXP):
            S = sizes[sl]
            t0 = offs[sl]
            ch = _chunks(S)
            hbig = hpool.tile([P, KO2 * S], BF16, tag="ht")
            for fo in range(FO1):
                w1t = w1pool.tile([P, KO1, P], BF16, tag="w1t")
                nc.sync.dma_start(w1t[:], w1s[sl, fo])
                for a, b in ch:
                    pa = psA.tile([P, 512], F32, tag="psA")
                    for ko in range(KO1):
                        nc.tensor.matmul(
                            pa[:, : b - a],
                            lhsT=w1t[:, ko, :],
                            rhs=xTe_s[:, ko, t0 + a : t0 + b],
                            start=(ko == 0),
                            stop=(ko == KO1 - 1),
                        )
                    nc.scalar.activation(
                        hbig[:, fo * S + a : fo * S + b],
                        pa[:, : b - a],
                        GELU,
                        bias=b1s_s[:, sl, fo : fo + 1],
                    )
            if sl + 1 < EXP:
                fetch_xte(sl + 1)  # prefetch after this slot's w1 stream
            for fo2 in range(FO2):
                w2t = w2pool.tile([P, KO2, P], BF16, tag="w2t")
                nc.sync.dma_start(w2t[:], w2s[sl, fo2])
                for a, b in ch:
                    pb = psB.tile([P, 512], F32, tag="psB")
                    for ko in range(KO2):
                        nc.tensor.matmul(
                            pb[:, : b - a],
                            lhsT=w2t[:, ko, :],
                            rhs=hbig[:, ko * S + a : ko * S + b],
                            start=(ko == 0),
                            stop=(ko == KO2 - 1),
                        )
                    # device returns tanh(0.5*o + 0.5*b2) in bf16; the host
                    # applies sigmoid = 0.5 + 0.5*tanh during scatter-add
                    # (b2s is pre-halved)
                    ot = opool.tile([P, 512], BF16, tag="ot")
                    nc.scalar.activation(
                        ot[:, : b - a],
                        pb[:, : b - a],
                        AF.Tanh,
                        bias=b2s_s[:, sl, fo2 : fo2 + 1],
                        scale=0.5,
                    )
                    nc.sync.dma_start(oT[:, fo2, t0 + a : t0 + b], ot[:, : b - a])
    nc.compile()
    return nc


# ======================================================================
# Host side
# ======================================================================

_NC_CACHE = {}


def prep_shared(inputs):
    """Host-side relayout of the shared (replicated) tensors."""
    bf16 = mybir.dt.np(BF16)
    gate_w1 = np.asarray(inputs["gate_w1"], np.float32)
    gate_b1 = np.asarray(inputs["gate_b1"], np.float32)
    gate_w2 = np.asarray(inputs["gate_w2"], np.float32)
    ew1 = np.asarray(inputs["ew1"], np.float32)
    eb1 = np.asarray(inputs["eb1"], np.float32)
    ew2 = np.asarray(inputs["ew2"], np.float32)
    eb2 = np.asarray(inputs["eb2"], np.float32)

    return {
        # [P, GFO, KO1, P]: gw1[d, f] -> [p_k, fo, ko, p_f]
        "gw1": np.ascontiguousarray(
            gate_w1.reshape(KO1, P, GFO, P).transpose(1, 2, 0, 3)
        ).astype(bf16),
        "gb1": np.ascontiguousarray(gate_b1.reshape(GFO, P).T),
        "gw2": np.ascontiguousarray(
            gate_w2.reshape(GFO, P, NE).transpose(1, 0, 2)
        ).astype(bf16),
        "w1e": np.ascontiguousarray(
            ew1.reshape(NE, KO1, P, FO1, P).transpose(0, 3, 2, 1, 4)
        ).astype(bf16),
        "b1e": np.ascontiguousarray(eb1.reshape(NE, FO1, P).transpose(2, 0, 1)),
        "w2e": np.ascontiguousarray(
            ew2.reshape(NE, KO2, P, FO2, P).transpose(0, 3, 2, 1, 4)
        ).astype(bf16),
        "b2e": np.ascontiguousarray(
            (0.5 * eb2).reshape(NE, FO2, P).transpose(2, 0, 1)
        ),
    }


def prep_xTb(inputs):
    bf16 = mybir.dt.np(BF16)
    combined = np.asarray(inputs["combined"], np.float32)
    xTbs = []
    for c in range(NCORES):
        xt = np.ascontiguousarray(
            combined[c * T : (c + 1) * T].T.reshape(KO1, P, T).transpose(1, 0, 2)
        ).astype(bf16)
        xTbs.append(xt)
    return xTbs


def _host_gelu(x):
    try:
        from scipy.special import erf
    except ImportError:  # pragma: no cover
        import math

        _erf_u = np.frompyfunc(math.erf, 1, 1)

        def erf(v):
            return _erf_u(v).astype(v.dtype)

    return (0.5 * x * (1.0 + erf(x / np.sqrt(np.float32(2.0))))).astype(np.float32)


def fixup_logits(logits, inputs):
    """Recompute exact fp32 logits for tokens whose 2-vs-3 margin is unsafe."""
    srt = np.sort(logits, axis=1)
    margin = srt[:, -2] - srt[:, -3]
    unsafe = np.nonzero(margin < MARGIN_THR)[0]
    if len(unsafe) == 0:
        return logits
    c = np.asarray(inputs["combined"], np.float32)[unsafe]
    gh = _host_gelu(
        c @ np.asarray(inputs["gate_w1"], np.float32)
        + np.asarray(inputs["gate_b1"], np.float32)
    )
    lg = gh @ np.asarray(inputs["gate_w2"], np.float32) + np.asarray(
        inputs["gate_b2"], np.float32
    )
    logits = logits.copy()
    logits[unsafe] = lg
    return logits


def _mm_cost(S):
    """ns of PE issue time per (fo,ko) weight tile at slot capacity S:
    per chunk max(stream, ~100ns LDWEIGHTS floor)."""
    return sum(max((b - a) / 2.4 + 2.5, 100.0) for a, b in _chunks(S))


def _assign(sizes, cnt_desc):
    """Greedy bin-cover: experts (desc counts) onto NCORES bins per class.
    Returns per-class lists of (expert, amount) or None if infeasible."""
    avail = [NCORES] * len(sizes)
    cls_desc = sorted(range(len(sizes)), key=lambda j: -sizes[j])
    out = [[] for _ in sizes]
    for e, ce in cnt_desc:
        rem = ce
        while rem > 0:
            jbig = next((j for j in cls_desc if avail[j] > 0), None)
            if jbig is None:
                return None
            if rem > sizes[jbig]:
                avail[jbig] -= 1
                out[jbig].append((e, sizes[jbig]))
                rem -= sizes[jbig]
            else:
                cands = [j for j in cls_desc if avail[j] > 0 and sizes[j] >= rem]
                j = min(cands, key=lambda q: sizes[q]) if cands else jbig
                avail[j] -= 1
                out[j].append((e, rem))
                rem = 0
    return out


def _best_sizes(cnt):
    """Search desc tuples of EXP multiples of 32 minimizing total PE issue
    cost subject to bin-cover feasibility."""
    cnt_desc = sorted(enumerate(cnt), key=lambda q: -q[1])
    best = [None]
    nodes = [0]

    def rec(prefix, remaining, maxv):
        if nodes[0] > 500000:
            return
        nodes[0] += 1
        if remaining == 0:
            if _assign(list(prefix), cnt_desc) is not None:
                cc = sum(_mm_cost(s) for s in prefix)
                if best[0] is None or cc < best[0][0]:
                    best[0] = (cc, tuple(prefix))
            return
        need = (sum(cnt) + NCORES - 1) // NCORES
        for v in range(min(maxv, 1024), 31, -32):
            if sum(prefix) + v * remaining < need:
                return
            pc = sum(_mm_cost(s) for s in prefix) + _mm_cost(v)
            if best[0] and pc + (remaining - 1) * 100.0 >= best[0][0]:
                continue
            rec(prefix + (v,), remaining - 1, v)

    rec(tuple(), EXP, 1024)
    if best[0] is None:  # fallback: uniform worst-case split
        S = max(32, int((max(cnt) + 31) // 32 * 32))
        return (min(S, 1024),) * EXP
    return best[0][1]


def route(logits):
    """Host softmax/top-2/normalize + slot assignment.

    NCORES*EXP slots in EXP size classes of NCORES bins each (core c runs
    one slot of every class, so all cores execute the same heterogeneous
    capacity tuple).  Class sizes are chosen by `_best_sizes` to minimize
    PE issue time subject to covering each expert's token count; the first
    slot is the smallest class so the initial xTe DMA lands fast.

    Returns (slots, sizes) where slots[c][j] = (expert, tokens, weights)
    for core c's j-th slot (len(tokens) <= sizes[j])."""
    lg = logits.astype(np.float32)
    m = lg.max(axis=1, keepdims=True)
    p = np.exp(lg - m)
    p /= p.sum(axis=1, keepdims=True)
    order = np.argsort(-p, axis=1, kind="stable")
    i1, i2 = order[:, 0], order[:, 1]
    r = np.arange(lg.shape[0])
    w1 = p[r, i1]
    w2 = p[r, i2]
    s = w1 + w2
    w1, w2 = w1 / s, w2 / s

    toks, wts = [], []
    for e in range(NE):
        t1 = np.nonzero(i1 == e)[0]
        t2 = np.nonzero(i2 == e)[0]
        toks.append(np.concatenate([t1, t2]))
        wts.append(np.concatenate([w1[t1], w2[t2]]).astype(np.float32))
    cnt = [len(t) for t in toks]
    cnt_desc = sorted(enumerate(cnt), key=lambda q: -q[1])

    sizes = sorted(_best_sizes(cnt), reverse=True)
    # mid-size slot first (fast xTe(0) landing), largest second (runs while
    # later prefetches have slack), smallest last (cheap tail; its weight
    # stream no longer competes with xTe prefetches)
    if len(sizes) > 1:
        sizes = (sizes[1], sizes[0]) + tuple(sizes[2:])
    sizes = tuple(sizes)
    percls = _assign(list(sizes), cnt_desc)
    assert percls is not None

    consumed = [0] * NE
    # per class j, bins percls[j] (<= NCORES) distributed one per core
    grid = [[(0, np.zeros(0, np.int64), np.zeros(0, np.float32))] * EXP
            for _ in range(NCORES)]
    for j in range(EXP):
        for c, (e, amt) in enumerate(percls[j]):
            a = consumed[e]
            consumed[e] += amt
            grid[c][j] = (e, toks[e][a : a + amt], wts[e][a : a + amt])
    assert consumed == cnt
    return grid, sizes


def kernel_sparse(**inputs):
    bf16 = mybir.dt.np(BF16)
    shared = prep_shared(inputs)
    xTbs = prep_xTb(inputs)

    if "gate" not in _NC_CACHE:
        _NC_CACHE["gate"] = build_nc_gate()
    ncg = _NC_CACHE["gate"]
    gmaps = [
        {
            "xTb": xTbs[c],
            "gw1": shared["gw1"],
            "gb1": shared["gb1"],
            "gw2": shared["gw2"],
        }
        for c in range(NCORES)
    ]
    gres = run_bass_kernel_spmd(ncg, gmaps, core_ids=list(range(NCORES)))
    logits = np.concatenate(
        [gres.results[c]["lgT"].T for c in range(NCORES)]
    )  # [N, NE]
    logits = logits + np.asarray(inputs["gate_b2"], np.float32)
    logits = fixup_logits(logits, inputs)

    slots, sizes = route(logits)
    Tc = sum(sizes)
    if Tc > SUMS_MAX:  # extremely unbalanced routing: use the dense fallback
        return kernel_dense(**inputs)
    offs = [0]
    for s in sizes:
        offs.append(offs[-1] + s)

    if ("exp", sizes) not in _NC_CACHE:
        _NC_CACHE[("exp", sizes)] = build_nc_exp(sizes)
    nce = _NC_CACHE[("exp", sizes)]

    combined = np.asarray(inputs["combined"], np.float32)
    emaps = []
    for c in range(NCORES):
        eids = [slots[c][j][0] for j in range(EXP)]
        xg = np.zeros((Tc, D), np.float32)
        for j in range(EXP):
            tk = slots[c][j][1]
            xg[offs[j] : offs[j] + len(tk)] = combined[tk]
        emaps.append(
            {
                "xTe": np.ascontiguousarray(
                    xg.T.reshape(KO1, P, Tc).transpose(1, 0, 2)
                ).astype(bf16),
                "w1s": np.ascontiguousarray(shared["w1e"][eids]),
                "b1s": np.ascontiguousarray(shared["b1e"][:, eids, :]),
                "w2s": np.ascontiguousarray(shared["w2e"][eids]),
                "b2s": np.ascontiguousarray(shared["b2e"][:, eids, :]),
            }
        )
    _NC_CACHE["last_emaps"] = emaps
    eres = run_bass_kernel_spmd(nce, emaps, core_ids=list(range(NCORES)))

    fused = np.zeros((N, E), np.float32)
    for c in range(NCORES):
        # device returns tanh(o/2 + b2/2); sigmoid = 0.5 + 0.5*tanh
        rows = (
            eres.results[c]["oT"].astype(np.float32).transpose(2, 1, 0).reshape(Tc, E)
        )
        rows = 0.5 + 0.5 * rows
        for j in range(EXP):
            _, tk, wt = slots[c][j]
            np.add.at(
                fused,
                tk,
                wt[:, None] * rows[offs[j] : offs[j] + len(tk)],
            )
    return fused


# ======================================================================
# Dense fallback (every expert on every token; no routing dependence).
# Only used if routing is so unbalanced that S > S_MAX.
# ======================================================================


def build_nc_dense():
    nc = bacc.Bacc("TRN2", target_bir_lowering=False, debug=False, num_devices=NCORES)
    xTb = nc.dram_tensor("xTb", [P, KO1, T], BF16, kind="ExternalInput").ap()
    gw1 = nc.dram_tensor("gw1", [P, GFO, KO1, P], BF16, kind="ExternalInput").ap()
    gb1 = nc.dram_tensor("gb1", [P, GFO], F32, kind="ExternalInput").ap()
    gw2 = nc.dram_tensor("gw2", [P, GFO, NE], BF16, kind="ExternalInput").ap()
    w1e = nc.dram_tensor("w1e", [NE, FO1, P, KO1, P], BF16, kind="ExternalInput").ap()
    b1e = nc.dram_tensor("b1e", [P, NE, FO1], F32, kind="ExternalInput").ap()
    w2e = nc.dram_tensor("w2e", [NE, FO2, P, KO2, P], BF16, kind="ExternalInput").ap()
    b2e = nc.dram_tensor("b2e", [P, NE, FO2], F32, kind="ExternalInput").ap()
    lgT = nc.dram_tensor("lgT", [NE, T], F32, kind="ExternalOutput").ap()
    eoT = nc.dram_tensor("eoT", [NE, P, FO2, T], BF16, kind="ExternalOutput").ap()

    import contextlib

    with tile.TileContext(nc) as tc, contextlib.ExitStack() as ctx:
        pers = ctx.enter_context(tc.tile_pool(name="pers", bufs=1))
        xTb_s = pers.tile([P, KO1, T], BF16)
        nc.sync.dma_start(xTb_s[:], xTb)
        b1e_s = pers.tile([P, NE, FO1], F32)
        nc.sync.dma_start(b1e_s[:], b1e)
        b2e_s = pers.tile([P, NE, FO2], F32)
        nc.sync.dma_start(b2e_s[:], b2e)

        # gate
        with (
            tc.tile_pool(name="gsb", bufs=1) as gsb,
            tc.tile_pool(name="glg", bufs=2) as glg,
            tc.tile_pool(name="gps", bufs=2, space="PSUM") as gps,
            tc.tile_pool(name="gpl", bufs=2, space="PSUM") as gpl,
        ):
            gw1_s = gsb.tile([P, GFO, KO1, P], BF16)
            nc.sync.dma_start(gw1_s[:], gw1)
            gb1_s = gsb.tile([P, GFO], F32)
            nc.sync.dma_start(gb1_s[:], gb1)
            gw2_s = gsb.tile([P, GFO, NE], BF16)
            nc.sync.dma_start(gw2_s[:], gw2)
            ghT = gsb.tile([P, GFO, T], BF16)
            for t2 in range(T // 512):
                sl = slice(t2 * 512, (t2 + 1) * 512)
                for fo in range(GFO):
                    pg = gps.tile([P, 512], F32, tag="pg")
                    for ko in range(KO1):
                        nc.tensor.matmul(
                            pg[:],
                            lhsT=gw1_s[:, fo, ko, :],
                            rhs=xTb_s[:, ko, sl],
                            start=(ko == 0),
                            stop=(ko == KO1 - 1),
                        )
                    nc.scalar.activation(
                        ghT[:, fo, sl], pg[:], GELU, bias=gb1_s[:, fo : fo + 1]
                    )
                pl = gpl.tile([NE, 512], F32, tag="pl")
                for fo in range(GFO):
                    nc.tensor.matmul(
                        pl[:],
                        lhsT=gw2_s[:, fo, :],
                        rhs=ghT[:, fo, sl],
                        start=(fo == 0),
                        stop=(fo == GFO - 1),
                    )
                lt = glg.tile([NE, 512], F32, tag="lt")
                nc.vector.tensor_copy(lt[:], pl[:])
                nc.sync.dma_start(lgT[:, sl], lt[:])

        # experts (dense)
        w1pool = ctx.enter_context(tc.tile_pool(name="w1p", bufs=4))
        w2pool = ctx.enter_context(tc.tile_pool(name="w2p", bufs=3))
        hpool = ctx.enter_context(tc.tile_pool(name="hp", bufs=1))
        spool = ctx.enter_context(tc.tile_pool(name="sp", bufs=2))
        opool = ctx.enter_context(tc.tile_pool(name="op", bufs=2))
        psA = ctx.enter_context(tc.tile_pool(name="psA", bufs=3, space="PSUM"))
        psB = ctx.enter_context(tc.tile_pool(name="psB", bufs=3, space="PSUM"))
        ch = _chunks(T)
        for e in range(NE):
            hbig = hpool.tile([P, KO2 * T], BF16, tag="ht")
            for fo in range(FO1):
                w1t = w1pool.tile([P, KO1, P], BF16, tag="w1t")
                nc.sync.dma_start(w1t[:], w1e[e, fo])
                for a, b in ch:
                    pa = psA.tile([P, 512], F32, tag="psA")
                    for ko in range(KO1):
                        nc.tensor.matmul(
                            pa[:, : b - a],
                            lhsT=w1t[:, ko, :],
                            rhs=xTb_s[:, ko, a:b],
                            start=(ko == 0),
                            stop=(ko == KO1 - 1),
                        )
                    nc.scalar.activation(
                        hbig[:, fo * T + a : fo * T + b],
                        pa[:, : b - a],
                        GELU,
                        bias=b1e_s[:, e, fo : fo + 1],
                    )
            for fo2 in range(FO2):
                w2t = w2pool.tile([P, KO2, P], BF16, tag="w2t")
                nc.sync.dma_start(w2t[:], w2e[e, fo2])
                for a, b in ch:
                    pb = psB.tile([P, 512], F32, tag="psB")
                    for ko in range(KO2):
                        nc.tensor.matmul(
                            pb[:, : b - a],
                            lhsT=w2t[:, ko, :],
                            rhs=hbig[:, ko * T + a : ko * T + b],
                            start=(ko == 0),
                            stop=(ko == KO2 - 1),
                        )
                    st = spool.tile([P, 512], F32, tag="st")
                    nc.scalar.activation(
                        st[:, : b - a],
                        pb[:, : b - a],
                        AF.Tanh,
                        bias=b2e_s[:, e, fo2 : fo2 + 1],
                        scale=0.5,
                    )
                    ot = opool.tile([P, 512], BF16, tag="ot")
                    nc.vector.tensor_scalar(
                        ot[:, : b - a], st[:, : b - a], 0.5, 0.5, OP.mult, OP.add
                    )
                    nc.sync.dma_start(eoT[e, :, fo2, a:b], ot[:, : b - a])
    nc.compile()
    return nc


def kernel_dense(**inputs):
    if "dense" not in _NC_CACHE:
        _NC_CACHE["dense"] = build_nc_dense()
    nc = _NC_CACHE["dense"]
    shared = prep_shared(inputs)
    xTbs = prep_xTb(inputs)
    in_maps = [
        {
            "xTb": xTbs[c],
            "gw1": shared["gw1"],
            "gb1": shared["gb1"],
            "gw2": shared["gw2"],
            "w1e": shared["w1e"],
            "b1e": shared["b1e"],
            "w2e": shared["w2e"],
            "b2e": shared["b2e"],
        }
        for c in range(NCORES)
    ]
    res = run_bass_kernel_spmd(nc, in_maps, core_ids=list(range(NCORES)))

    logits = np.concatenate([res.results[c]["lgT"].T for c in range(NCORES)])
    logits = logits + np.asarray(inputs["gate_b2"], np.float32)
    logits = fixup_logits(logits, inputs)
    lg = logits
    m = lg.max(axis=1, keepdims=True)
    p = np.exp(lg - m)
    p /= p.sum(axis=1, keepdims=True)
    order = np.argsort(-p, axis=1, kind="stable")
    i1, i2 = order[:, 0], order[:, 1]
    r = np.arange(lg.shape[0])
    w1 = p[r, i1]
    w2 = p[r, i2]
    s = w1 + w2
    w1, w2 = w1 / s, w2 / s

    fused = np.zeros((N, E), np.float32)
    for c in range(NCORES):
        eo = res.results[c]["eoT"].astype(np.float32)  # [NE, P, FO2, T]
        eo = eo.transpose(0, 3, 2, 1).reshape(NE, T, E)  # [NE, T, E]
        rr = np.arange(c * T, (c + 1) * T)
        fused[rr] += w1[rr, None] * eo[i1[rr], np.arange(T)]
        fused[rr] += w2[rr, None] * eo[i2[rr], np.arange(T)]
    return fused


MODE = "sparse"


def _spot_check(out, inputs, ntok=4, tol=5e-2):
    """Recompute a few tokens exactly on host; reject corrupted device runs."""
    try:
        idx = np.arange(0, N, N // ntok)[:ntok]
        c = np.asarray(inputs["combined"], np.float32)[idx]
        gh = _host_gelu(
            c @ np.asarray(inputs["gate_w1"], np.float32)
            + np.asarray(inputs["gate_b1"], np.float32)
        )
        lg = gh @ np.asarray(inputs["gate_w2"], np.float32) + np.asarray(
            inputs["gate_b2"], np.float32
        )
        p = np.exp(lg - lg.max(axis=1, keepdims=True))
        p /= p.sum(axis=1, keepdims=True)
        order = np.argsort(-p, axis=1, kind="stable")
        ew1 = np.asarray(inputs["ew1"], np.float32)
        eb1 = np.asarray(inputs["eb1"], np.float32)
        ew2 = np.asarray(inputs["ew2"], np.float32)
        eb2 = np.asarray(inputs["eb2"], np.float32)
        for t in range(ntok):
            i1, i2 = int(order[t, 0]), int(order[t, 1])
            w1 = p[t, i1] / (p[t, i1] + p[t, i2])
            exp_row = np.zeros(E, np.float32)
            for e, w in ((i1, w1), (i2, 1.0 - w1)):
                h = _host_gelu(c[t] @ ew1[e] + eb1[e])
                o = 1.0 / (1.0 + np.exp(-(h @ ew2[e] + eb2[e])))
                exp_row += w * o
            if not np.isfinite(out[idx[t]]).all():
                return False
            if np.abs(out[idx[t]] - exp_row).max() > tol:
                return False
        return True
    except Exception:
        return True  # never let the checker itself kill a good run


def kernel(**inputs):
    best = None
    if MODE == "sparse":
        for _ in range(3):  # transient device errors usually recover on retry
            try:
                out = kernel_sparse(**inputs)
            except Exception:
                continue
            if _spot_check(out, inputs):
                return out
            best = out
    try:
        out = kernel_dense(**inputs)
        if _spot_check(out, inputs) or best is None:
            return out
    except Exception:
        pass
    if best is not None:
        return best
    return kernel_dense(**inputs)


if __name__ == "__main__":  # dev smoke test only; harness imports kernel()
    import reference  # noqa: PLC0415 -- not needed when imported as a module

    inputs = {k: np.asarray(v) for k, v in reference.setup_inputs().items()}
    out = kernel(**inputs)
    print(out.shape, out.dtype)


# revision 27
# speedup vs baseline: 1.0078x; 1.0078x over previous
"""MoE fusion kernel for Trainium2 (8 NeuronCores, two-phase sparse routing).

Phase 1 (gate NEFF, all-bf16): computes gate logits on device.  bf16 logit
error is <= ~1e-2; any token whose 2nd-vs-3rd logit margin is below
MARGIN_THR is recomputed exactly (fp32) on the host before routing, so the
top-2 selection always matches the fp32 reference.  The combine weights
collapse to sigmoid(l1 - l2), which tolerates ~1e-2 logit error harmlessly.

Phase 2 (expert NEFF, bf16): host routes each token's top-2 experts into
24 uniform slots (3 per core, capacity S).  Each slot runs one expert's
Linear-GELU-Linear over its gathered tokens in a feature-major layout:

    x.T [1536, S] -> h.T = gelu(W1.T x.T) [3072, S] -> o.T = W2.T h.T [768, S]

Matmuls are weights-stationary with 512-column moving chunks (PSUM bank
limit); each matmul instruction costs ~N/2.4 + 43 ns, so big-N chunks and
low padding dominate the schedule.  Device returns sigmoid(o) in bf16; the
host applies the combine weights during scatter-add.
"""

import numpy as np

try:
    import concourse  # noqa: F401
except ImportError:  # pragma: no cover
    import sys

    sys.path.insert(0, "/opt/trn_rl_repo")

import concourse.bass as bass  # noqa: F401
import concourse.mybir as mybir
import concourse.tile as tile
from concourse import bacc
from concourse.bass_utils import run_bass_kernel_spmd

# Problem shapes (hardcoded per contest rules).
N, D, E, H, NE = 8192, 1536, 768, 3072, 12
NCORES = 8
T = N // NCORES  # 1024 tokens per core
P = 128
KO1 = D // P  # 12   k-tiles of the first expert matmul
FO1 = H // P  # 24   feature-tiles of h
KO2 = H // P  # 24   k-tiles of the second expert matmul
FO2 = E // P  # 6    feature-tiles of the output
GFO = E // P  # 6    feature-tiles of the gate hidden

F32 = mybir.dt.float32
BF16 = mybir.dt.bfloat16
AF = mybir.ActivationFunctionType
OP = mybir.AluOpType

GELU = AF.Gelu  # test.py sim-mode substitutes Tanh (CoreSim lacks Gelu)

EXP = 5  # expert slots per core (8*EXP slots total, assigned by load)
SUMS_MAX = 4096  # cap on per-core total slot capacity (SBUF) -> dense fallback
MARGIN_THR = 0.03  # host-recompute tokens with 2nd-3rd logit margin below this


def _chunks(total):
    """Column chunks <=512 (PSUM bank limit); even split for 512<S<=1024 so
    neither chunk drops under the ~100ns LDWEIGHTS issue floor."""
    if total <= 512:
        return [(0, total)]
    assert total <= 1024
    h = (total + 1) // 2
    return [(0, h), (h, total)]


# ======================================================================
# Phase-1 NEFF: bf16 gate -> logits.T [NE, T] per core
# ======================================================================


def build_nc_gate():
    nc = bacc.Bacc("TRN2", target_bir_lowering=False, debug=False, num_devices=NCORES)
    xTb = nc.dram_tensor("xTb", [P, KO1, T], BF16, kind="ExternalInput").ap()
    gw1 = nc.dram_tensor("gw1", [P, GFO, KO1, P], BF16, kind="ExternalInput").ap()
    gb1 = nc.dram_tensor("gb1", [P, GFO], F32, kind="ExternalInput").ap()
    gw2 = nc.dram_tensor("gw2", [P, GFO, NE], BF16, kind="ExternalInput").ap()
    lgT = nc.dram_tensor("lgT", [NE, T], F32, kind="ExternalOutput").ap()

    with tile.TileContext(nc) as tc:
        with (
            tc.tile_pool(name="sb", bufs=1) as sb,
            tc.tile_pool(name="lg", bufs=2) as lg,
            tc.tile_pool(name="ps", bufs=2, space="PSUM") as ps,
            tc.tile_pool(name="pl", bufs=2, space="PSUM") as pls,
        ):
            gw1_s = sb.tile([P, GFO, KO1, P], BF16)
            xTb_s = sb.tile([P, KO1, T], BF16)
            gb1_s = sb.tile([P, GFO], F32)
            gw2_s = sb.tile([P, GFO, NE], BF16)
            # interleave so that (gw1[fo0], xTb[t2=0]) land first; one strided
            # DMA per t2-chunk (piecewise issue serializes on the Sync engine)
            nc.sync.dma_start(gw1_s[:, 0], gw1[:, 0])
            nc.sync.dma_start(xTb_s[:, :, 0:512], xTb[:, :, 0:512])
            nc.sync.dma_start(gb1_s[:], gb1)
            nc.sync.dma_start(gw2_s[:], gw2)
            for fo in range(1, GFO):
                nc.sync.dma_start(gw1_s[:, fo], gw1[:, fo])
            for t2 in range(1, T // 512):
                nc.sync.dma_start(
                    xTb_s[:, :, t2 * 512 : (t2 + 1) * 512],
                    xTb[:, :, t2 * 512 : (t2 + 1) * 512],
                )
            ghT = sb.tile([P, GFO, T], BF16)

            for t2 in range(T // 512):
                sl = slice(t2 * 512, (t2 + 1) * 512)
                for fo in range(GFO):
                    pg = ps.tile([P, 512], F32, tag="pg")
                    for ko in range(KO1):
                        nc.tensor.matmul(
                            pg[:],
                            lhsT=gw1_s[:, fo, ko, :],
                            rhs=xTb_s[:, ko, sl],
                            start=(ko == 0),
                            stop=(ko == KO1 - 1),
                        )
                    nc.scalar.activation(
                        ghT[:, fo, sl], pg[:], GELU, bias=gb1_s[:, fo : fo + 1]
                    )
                pl = pls.tile([NE, 512], F32, tag="pl")
                for fo in range(GFO):
                    nc.tensor.matmul(
                        pl[:],
                        lhsT=gw2_s[:, fo, :],
                        rhs=ghT[:, fo, sl],
                        start=(fo == 0),
                        stop=(fo == GFO - 1),
                    )
                lt = lg.tile([NE, 512], F32, tag="lt")
                nc.vector.tensor_copy(lt[:], pl[:])
                nc.sync.dma_start(lgT[:, sl], lt[:])
    nc.compile()
    return nc


# ======================================================================
# Phase-2 NEFF: per-core EXP expert slots with per-slot capacities `sizes`
# (identical across cores; slot loads are classed by the host router).
# ======================================================================


def build_nc_exp(sizes):
    sizes = tuple(int(s) for s in sizes)
    Tc = sum(sizes)
    offs = [0]
    for s in sizes:
        offs.append(offs[-1] + s)
    nc = bacc.Bacc("TRN2", target_bir_lowering=False, debug=False, num_devices=NCORES)
    xTe = nc.dram_tensor("xTe", [P, KO1, Tc], BF16, kind="ExternalInput").ap()
    w1s = nc.dram_tensor("w1s", [EXP, FO1, P, KO1, P], BF16, kind="ExternalInput").ap()
    b1s = nc.dram_tensor("b1s", [P, EXP, FO1], F32, kind="ExternalInput").ap()
    w2s = nc.dram_tensor("w2s", [EXP, FO2, P, KO2, P], BF16, kind="ExternalInput").ap()
    b2s = nc.dram_tensor("b2s", [P, EXP, FO2], F32, kind="ExternalInput").ap()
    oT = nc.dram_tensor("oT", [P, FO2, Tc], BF16, kind="ExternalOutput").ap()

    import contextlib

    with tile.TileContext(nc) as tc, contextlib.ExitStack() as ctx:
        pers = ctx.enter_context(tc.tile_pool(name="pers", bufs=1))
        b1s_s = pers.tile([P, EXP, FO1], F32)
        b2s_s = pers.tile([P, EXP, FO2], F32)
        xTe_s = pers.tile([P, KO1, Tc], BF16)

        def fetch_xte(sl):
            # one strided DMA for the whole slot (DMA-issue on Sync costs
            # ~0.65us each; piecewise issue serializes the startup)
            nc.sync.dma_start(
                xTe_s[:, :, offs[sl] : offs[sl + 1]],
                xTe[:, :, offs[sl] : offs[sl + 1]],
            )

        fetch_xte(0)  # slot 0's tokens land first; later slots prefetch below
        nc.sync.dma_start(b1s_s[:], b1s)  # biases after: consumed only by ACT
        nc.sync.dma_start(b2s_s[:], b2s)

        w1pool = ctx.enter_context(tc.tile_pool(name="w1p", bufs=6))
        w2pool = ctx.enter_context(tc.tile_pool(name="w2p", bufs=4))
        hpool = ctx.enter_context(tc.tile_pool(name="hp", bufs=1))
        spool = ctx.enter_context(tc.tile_pool(name="sp", bufs=4))
        opool = ctx.enter_context(tc.tile_pool(name="op", bufs=4))
        psA = ctx.enter_context(tc.tile_pool(name="psA", bufs=4, space="PSUM"))
        psB = ctx.enter_context(tc.tile_pool(name="psB", bufs=4, space="PSUM"))

        for sl in range(EXP):
            S = sizes[sl]
            t0 = offs[sl]
            ch = _chunks(S)
            hbig = hpool.tile([P, KO2 * S], BF16, tag="ht")
            for fo in range(FO1):
                w1t = w1pool.tile([P, KO1, P], BF16, tag="w1t")
                nc.sync.dma_start(w1t[:], w1s[sl, fo])
                for a, b in ch:
                    pa = psA.tile([P, 512], F32, tag="psA")
                    for ko in range(KO1):
                        nc.tensor.matmul(
                            pa[:, : b - a],
                            lhsT=w1t[:, ko, :],
                            rhs=xTe_s[:, ko, t0 + a : t0 + b],
                            start=(ko == 0),
                            stop=(ko == KO1 - 1),
                        )
                    nc.scalar.activation(
                        hbig[:, fo * S + a : fo * S + b],
                        pa[:, : b - a],
                        GELU,
                        bias=b1s_s[:, sl, fo : fo + 1],
                    )
            if sl + 1 < EXP:
                fetch_xte(sl + 1)  # prefetch after this slot's w1 stream
            for fo2 in range(FO2):
                w2t = w2pool.tile([P, KO2, P], BF16, tag="w2t")
                nc.sync.dma_start(w2t[:], w2s[sl, fo2])
                for a, b in ch:
                    pb = psB.tile([P, 512], F32, tag="psB")
                    for ko in range(KO2):
                        nc.tensor.matmul(
                            pb[:, : b - a],
                            lhsT=w2t[:, ko, :],
                            rhs=hbig[:, ko * S + a : ko * S + b],
                            start=(ko == 0),
                            stop=(ko == KO2 - 1),
                        )
                    # device returns tanh(0.5*o + 0.5*b2) in bf16; the host
                    # applies sigmoid = 0.5 + 0.5*tanh during scatter-add
                    # (b2s is pre-halved)
                    ot = opool.tile([P, 512], BF16, tag="ot")
                    nc.scalar.activation(
                        ot[:, : b - a],
                        pb[:, : b - a],
                        AF.Tanh,
                        bias=b2s_s[:, sl, fo2 : fo2 + 1],
                        scale=0.5,
                    )
                    nc.sync.dma_start(oT[:, fo2, t0 + a : t0 + b], ot[:, : b - a])
    nc.compile()
    return nc


# ======================================================================
# Host side
# ======================================================================

_NC_CACHE = {}


def prep_shared(inputs):
    """Host-side relayout of the shared (replicated) tensors."""
    bf16 = mybir.dt.np(BF16)
    gate_w1 = np.asarray(inputs["gate_w1"], np.float32)
    gate_b1 = np.asarray(inputs["gate_b1"], np.float32)
    gate_w2 = np.asarray(inputs["gate_w2"], np.float32)
    ew1 = np.asarray(inputs["ew1"], np.float32)
    eb1 = np.asarray(inputs["eb1"], np.float32)
    ew2 = np.asarray(inputs["ew2"], np.float32)
    eb2 = np.asarray(inputs["eb2"], np.float32)

    return {
        # [P, GFO, KO1, P]: gw1[d, f] -> [p_k, fo, ko, p_f]
        "gw1": np.ascontiguousarray(
            gate_w1.reshape(KO1, P, GFO, P).transpose(1, 2, 0, 3)
        ).astype(bf16),
        "gb1": np.ascontiguousarray(gate_b1.reshape(GFO, P).T),
        "gw2": np.ascontiguousarray(
            gate_w2.reshape(GFO, P, NE).transpose(1, 0, 2)
        ).astype(bf16),
        "w1e": np.ascontiguousarray(
            ew1.reshape(NE, KO1, P, FO1, P).transpose(0, 3, 2, 1, 4)
        ).astype(bf16),
        "b1e": np.ascontiguousarray(eb1.reshape(NE, FO1, P).transpose(2, 0, 1)),
        "w2e": np.ascontiguousarray(
            ew2.reshape(NE, KO2, P, FO2, P).transpose(0, 3, 2, 1, 4)
        ).astype(bf16),
        "b2e": np.ascontiguousarray(
            (0.5 * eb2).reshape(NE, FO2, P).transpose(2, 0, 1)
        ),
    }


def prep_xTb(inputs):
    bf16 = mybir.dt.np(BF16)
    combined = np.asarray(inputs["combined"], np.float32)
    xTbs = []
    for c in range(NCORES):
        xt = np.ascontiguousarray(
            combined[c * T : (c + 1) * T].T.reshape(KO1, P, T).transpose(1, 0, 2)
        ).astype(bf16)
        xTbs.append(xt)
    return xTbs


def _host_gelu(x):
    try:
        from scipy.special import erf
    except ImportError:  # pragma: no cover
        import math

        _erf_u = np.frompyfunc(math.erf, 1, 1)

        def erf(v):
            return _erf_u(v).astype(v.dtype)

    return (0.5 * x * (1.0 + erf(x / np.sqrt(np.float32(2.0))))).astype(np.float32)


def fixup_logits(logits, inputs):
    """Recompute exact fp32 logits for tokens whose 2-vs-3 margin is unsafe."""
    srt = np.sort(logits, axis=1)
    margin = srt[:, -2] - srt[:, -3]
    unsafe = np.nonzero(margin < MARGIN_THR)[0]
    if len(unsafe) == 0:
        return logits
    c = np.asarray(inputs["combined"], np.float32)[unsafe]
    gh = _host_gelu(
        c @ np.asarray(inputs["gate_w1"], np.float32)
        + np.asarray(inputs["gate_b1"], np.float32)
    )
    lg = gh @ np.asarray(inputs["gate_w2"], np.float32) + np.asarray(
        inputs["gate_b2"], np.float32
    )
    logits = logits.copy()
    logits[unsafe] = lg
    return logits


def _mm_cost(S):
    """ns of PE issue time per (fo,ko) weight tile at slot capacity S:
    per chunk max(stream, ~100ns LDWEIGHTS floor)."""
    return sum(max((b - a) / 2.4 + 2.5, 100.0) for a, b in _chunks(S))


def _assign(sizes, cnt_desc):
    """Greedy bin-cover: experts (desc counts) onto NCORES bins per class.
    Returns per-class lists of (expert, amount) or None if infeasible."""
    avail = [NCORES] * len(sizes)
    cls_desc = sorted(range(len(sizes)), key=lambda j: -sizes[j])
    out = [[] for _ in sizes]
    for e, ce in cnt_desc:
        rem = ce
        while rem > 0:
            jbig = next((j for j in cls_desc if avail[j] > 0), None)
            if jbig is None:
                return None
            if rem > sizes[jbig]:
                avail[jbig] -= 1
                out[jbig].append((e, sizes[jbig]))
                rem -= sizes[jbig]
            else:
                cands = [j for j in cls_desc if avail[j] > 0 and sizes[j] >= rem]
                j = min(cands, key=lambda q: sizes[q]) if cands else jbig
                avail[j] -= 1
                out[j].append((e, rem))
                rem = 0
    return out


def _best_sizes(cnt):
    """Search desc tuples of EXP multiples of 32 minimizing total PE issue
    cost subject to bin-cover feasibility."""
    cnt_desc = sorted(enumerate(cnt), key=lambda q: -q[1])
    best = [None]
    nodes = [0]

    def rec(prefix, remaining, maxv):
        if nodes[0] > 500000:
            return
        nodes[0] += 1
        if remaining == 0:
            if _assign(list(prefix), cnt_desc) is not None:
                cc = sum(_mm_cost(s) for s in prefix)
                if best[0] is None or cc < best[0][0]:
                    best[0] = (cc, tuple(prefix))
            return
        need = (sum(cnt) + NCORES - 1) // NCORES
        for v in range(min(maxv, 1024), 31, -32):
            if sum(prefix) + v * remaining < need:
                return
            pc = sum(_mm_cost(s) for s in prefix) + _mm_cost(v)
            if best[0] and pc + (remaining - 1) * 100.0 >= best[0][0]:
                continue
            rec(prefix + (v,), remaining - 1, v)

    rec(tuple(), EXP, 1024)
    if best[0] is None:  # fallback: uniform worst-case split
        S = max(32, int((max(cnt) + 31) // 32 * 32))
        return (min(S, 1024),) * EXP
    return best[0][1]


def route(logits):
    """Host softmax/top-2/normalize + slot assignment.

    NCORES*EXP slots in EXP size classes of NCORES bins each (core c runs
    one slot of every class, so all cores execute the same heterogeneous
    capacity tuple).  Class sizes are chosen by `_best_sizes` to minimize
    PE issue time subject to covering each expert's token count; the first
    slot is the smallest class so the initial xTe DMA lands fast.

    Returns (slots, sizes) where slots[c][j] = (expert, tokens, weights)
    for core c's j-th slot (len(tokens) <= sizes[j])."""
    lg = logits.astype(np.float32)
    m = lg.max(axis=1, keepdims=True)
    p = np.exp(lg - m)
    p /= p.sum(axis=1, keepdims=True)
    order = np.argsort(-p, axis=1, kind="stable")
    i1, i2 = order[:, 0], order[:, 1]
    r = np.arange(lg.shape[0])
    w1 = p[r, i1]
    w2 = p[r, i2]
    s = w1 + w2
    w1, w2 = w1 / s, w2 / s

    toks, wts = [], []
    for e in range(NE):
        t1 = np.nonzero(i1 == e)[0]
        t2 = np.nonzero(i2 == e)[0]
        toks.append(np.concatenate([t1, t2]))
        wts.append(np.concatenate([w1[t1], w2[t2]]).astype(np.float32))
    cnt = [len(t) for t in toks]
    cnt_desc = sorted(enumerate(cnt), key=lambda q: -q[1])

    sizes = sorted(_best_sizes(cnt), reverse=True)
    # mid-size slot first (fast xTe(0) landing), largest second (runs while
    # later prefetches have slack), smallest last (cheap tail; its weight
    # stream no longer competes with xTe prefetches)
    if len(sizes) > 1:
        sizes = (sizes[1], sizes[0]) + tuple(sizes[2:])
    sizes = tuple(sizes)
    percls = _assign(list(sizes), cnt_desc)
    assert percls is not None

    consumed = [0] * NE
    # per class j, bins percls[j] (<= NCORES) distributed one per core
    grid = [[(0, np.zeros(0, np.int64), np.zeros(0, np.float32))] * EXP
            for _ in range(NCORES)]
    for j in range(EXP):
        for c, (e, amt) in enumerate(percls[j]):
            a = consumed[e]
            consumed[e] += amt
            grid[c][j] = (e, toks[e][a : a + amt], wts[e][a : a + amt])
    assert consumed == cnt
    return grid, sizes


def kernel_sparse(**inputs):
    bf16 = mybir.dt.np(BF16)
    shared = prep_shared(inputs)
    xTbs = prep_xTb(inputs)

    if "gate" not in _NC_CACHE:
        _NC_CACHE["gate"] = build_nc_gate()
    ncg = _NC_CACHE["gate"]
    gmaps = [
        {
            "xTb": xTbs[c],
            "gw1": shared["gw1"],
            "gb1": shared["gb1"],
            "gw2": shared["gw2"],
        }
        for c in range(NCORES)
    ]
    gres = run_bass_kernel_spmd(ncg, gmaps, core_ids=list(range(NCORES)))
    logits = np.concatenate(
        [gres.results[c]["lgT"].T for c in range(NCORES)]
    )  # [N, NE]
    logits = logits + np.asarray(inputs["gate_b2"], np.float32)
    logits = fixup_logits(logits, inputs)

    slots, sizes = route(logits)
    Tc = sum(sizes)
    if Tc > SUMS_MAX:  # extremely unbalanced routing: use the dense fallback
        return kernel_dense(**inputs)
    offs = [0]
    for s in sizes:
        offs.append(offs[-1] + s)

    if ("exp", sizes) not in _NC_CACHE:
        _NC_CACHE[("exp", sizes)] = build_nc_exp(sizes)
    nce = _NC_CACHE[("exp", sizes)]

    combined = np.asarray(inputs["combined"], np.float32)
    emaps = []
    for c in range(NCORES):
        eids = [slots[c][j][0] for j in range(EXP)]
        xg = np.zeros((Tc, D), np.float32)
        for j in range(EXP):
            tk = slots[c][j][1]
            xg[offs[j] : offs[j] + len(tk)] = combined[tk]
        emaps.append(
            {
                "xTe": np.ascontiguousarray(
                    xg.T.reshape(KO1, P, Tc).transpose(1, 0, 2)
                ).astype(bf16),
                "w1s": np.ascontiguousarray(shared["w1e"][eids]),
                "b1s": np.ascontiguousarray(shared["b1e"][:, eids, :]),
                "w2s": np.ascontiguousarray(shared["w2e"][eids]),
                "b2s": np.ascontiguousarray(shared["b2e"][:, eids, :]),
            }
        )
    _NC_CACHE["last_emaps"] = emaps
    eres = run_bass_kernel_spmd(nce, emaps, core_ids=list(range(NCORES)))

    fused = np.zeros((N, E), np.float32)
    for c in range(NCORES):
        # device returns tanh(o/2 + b2/2); sigmoid = 0.5 + 0.5*tanh
        rows = (
            eres.results[c]["oT"].astype(np.float32).transpose(2, 1, 0).reshape(Tc, E)
        )
        rows = 0.5 + 0.5 * rows
        for j in range(EXP):
            _, tk, wt = slots[c][j]
            np.add.at(
                fused,
                tk,
                wt[:, None] * rows[offs[j] : offs[j] + len(tk)],
            )
    return fused


# ======================================================================
# Dense fallback (every expert on every token; no routing dependence).
# Only used if routing is so unbalanced that S > S_MAX.
# ======================================================================


def build_nc_dense():
    nc = bacc.Bacc("TRN2", target_bir_lowering=False, debug=False, num_devices=NCORES)
    xTb = nc.dram_tensor("xTb", [P, KO1, T], BF16, kind="ExternalInput").ap()
    gw1 = nc.dram_tensor("gw1", [P, GFO, KO1, P], BF16, kind="ExternalInput").ap()
    gb1 = nc.dram_tensor("gb1", [P, GFO], F32, kind="ExternalInput").ap()
    gw2 = nc.dram_tensor("gw2", [P, GFO, NE], BF16, kind="ExternalInput").ap()
    w1e = nc.dram_tensor("w1e", [NE, FO1, P, KO1, P], BF16, kind="ExternalInput").ap()
    b1e = nc.dram_tensor("b1e", [P, NE, FO1], F32, kind="ExternalInput").ap()
    w2e = nc.dram_tensor("w2e", [NE, FO2, P, KO2, P], BF16, kind="ExternalInput").ap()
    b2e = nc.dram_tensor("b2e", [P, NE, FO2], F32, kind="ExternalInput").ap()
    lgT = nc.dram_tensor("lgT", [NE, T], F32, kind="ExternalOutput").ap()
    eoT = nc.dram_tensor("eoT", [NE, P, FO2, T], BF16, kind="ExternalOutput").ap()

    import contextlib

    with tile.TileContext(nc) as tc, contextlib.ExitStack() as ctx:
        pers = ctx.enter_context(tc.tile_pool(name="pers", bufs=1))
        xTb_s = pers.tile([P, KO1, T], BF16)
        nc.sync.dma_start(xTb_s[:], xTb)
        b1e_s = pers.tile([P, NE, FO1], F32)
        nc.sync.dma_start(b1e_s[:], b1e)
        b2e_s = pers.tile([P, NE, FO2], F32)
        nc.sync.dma_start(b2e_s[:], b2e)

        # gate
        with (
            tc.tile_pool(name="gsb", bufs=1) as gsb,
            tc.tile_pool(name="glg", bufs=2) as glg,
            tc.tile_pool(name="gps", bufs=2, space="PSUM") as gps,
            tc.tile_pool(name="gpl", bufs=2, space="PSUM") as gpl,
        ):
            gw1_s = gsb.tile([P, GFO, KO1, P], BF16)
            nc.sync.dma_start(gw1_s[:], gw1)
            gb1_s = gsb.tile([P, GFO], F32)
            nc.sync.dma_start(gb1_s[:], gb1)
            gw2_s = gsb.tile([P, GFO, NE], BF16)
            nc.sync.dma_start(gw2_s[:], gw2)
            ghT = gsb.tile([P, GFO, T], BF16)
            for t2 in range(T // 512):
                sl = slice(t2 * 512, (t2 + 1) * 512)
                for fo in range(GFO):
                    pg = gps.tile([P, 512], F32, tag="pg")
                    for ko in range(KO1):
                        nc.tensor.matmul(
                            pg[:],
                            lhsT=gw1_s[:, fo, ko, :],
                            rhs=xTb_s[:, ko, sl],
                            start=(ko == 0),
                            stop=(ko == KO1 - 1),
                        )
                    nc.scalar.activation(
                        ghT[:, fo, sl], pg[:], GELU, bias=gb1_s[:, fo : fo + 1]
                    )
                pl = gpl.tile([NE, 512], F32, tag="pl")
                for fo in range(GFO):
                    nc.tensor.matmul(
                        pl[:],
                        lhsT=gw2_s[:, fo, :],
                        rhs=ghT[:, fo, sl],
                        start=(fo == 0),
                        stop=(fo == GFO - 1),
                    )
                lt = glg.tile([NE, 512], F32, tag="lt")
                nc.vector.tensor_copy(lt[:], pl[:])
                nc.sync.dma_start(lgT[:, sl], lt[:])

        # experts (dense)
        w1pool = ctx.enter_context(tc.tile_pool(name="w1p", bufs=4))
        w2pool = ctx.enter_context(tc.tile_pool(name="w2p", bufs=3))
        hpool = ctx.enter_context(tc.tile_pool(name="hp", bufs=1))
        spool = ctx.enter_context(tc.tile_pool(name="sp", bufs=2))
        opool = ctx.enter_context(tc.tile_pool(name="op", bufs=2))
        psA = ctx.enter_context(tc.tile_pool(name="psA", bufs=3, space="PSUM"))
        psB = ctx.enter_context(tc.tile_pool(name="psB", bufs=3, space="PSUM"))
        ch = _chunks(T)
        for e in range(NE):
            hbig = hpool.tile([P, KO2 * T], BF16, tag="ht")
            for fo in range(FO1):
                w1t = w1pool.tile([P, KO1, P], BF16, tag="w1t")
                nc.sync.dma_start(w1t[:], w1e[e, fo])
                for a, b in ch:
                    pa = psA.tile([P, 512], F32, tag="psA")
                    for ko in range(KO1):
                        nc.tensor.matmul(
                            pa[:, : b - a],
                            lhsT=w1t[:, ko, :],
                            rhs=xTb_s[:, ko, a:b],
                            start=(ko == 0),
                            stop=(ko == KO1 - 1),
                        )
                    nc.scalar.activation(
                        hbig[:, fo * T + a : fo * T + b],
                        pa[:, : b - a],
                        GELU,
                        bias=b1e_s[:, e, fo : fo + 1],
                    )
            for fo2 in range(FO2):
                w2t = w2pool.tile([P, KO2, P], BF16, tag="w2t")
                nc.sync.dma_start(w2t[:], w2e[e, fo2])
                for a, b in ch:
                    pb = psB.tile([P, 512], F32, tag="psB")
                    for ko in range(KO2):
                        nc.tensor.matmul(
                            pb[:, : b - a],
                            lhsT=w2t[:, ko, :],
                            rhs=hbig[:, ko * T + a : ko * T + b],
                            start=(ko == 0),
                            stop=(ko == KO2 - 1),
                        )
                    st = spool.tile([P, 512], F32, tag="st")
                    nc.scalar.activation(
                        st[:, : b - a],
                        pb[:, : b - a],
                        AF.Tanh,
                        bias=b2e_s[:, e, fo2 : fo2 + 1],
                        scale=0.5,
                    )
                    ot = opool.tile([P, 512], BF16, tag="ot")
                    nc.vector.tensor_scalar(
                        ot[:, : b - a], st[:, : b - a], 0.5, 0.5, OP.mult, OP.add
                    )
                    nc.sync.dma_start(eoT[e, :, fo2, a:b], ot[:, : b - a])
    nc.compile()
    return nc


def kernel_dense(**inputs):
    if "dense" not in _NC_CACHE:
        _NC_CACHE["dense"] = build_nc_dense()
    nc = _NC_CACHE["dense"]
    shared = prep_shared(inputs)
    xTbs = prep_xTb(inputs)
    in_maps = [
        {
            "xTb": xTbs[c],
            "gw1": shared["gw1"],
            "gb1": shared["gb1"],
            "gw2": shared["gw2"],
            "w1e": shared["w1e"],
            "b1e": shared["b1e"],
            "w2e": shared["w2e"],
            "b2e": shared["b2e"],
        }
        for c in range(NCORES)
    ]
    res = run_bass_kernel_spmd(nc, in_maps, core_ids=list(range(NCORES)))

    logits = np.concatenate([res.results[c]["lgT"].T for c in range(NCORES)])
    logits = logits + np.asarray(inputs["gate_b2"], np.float32)
    logits = fixup_logits(logits, inputs)
    lg = logits
    m = lg.max(axis=1, keepdims=True)
    p = np.exp(lg - m)
    p /= p.sum(axis=1, keepdims=True)
    order = np.argsort(-p, axis=1, kind="stable")
    i1, i2 = order[:, 0], order[:, 1]
    r = np.arange(lg.shape[0])
    w1 = p[r, i1]
    w2 = p[r, i2]
    s = w1 + w2
    w1, w2 = w1 / s, w2 / s

    fused = np.zeros((N, E), np.float32)
    for c in range(NCORES):
        eo = res.results[c]["eoT"].astype(np.float32)  # [NE, P, FO2, T]
        eo = eo.transpose(0, 3, 2, 1).reshape(NE, T, E)  # [NE, T, E]
        rr = np.arange(c * T, (c + 1) * T)
        fused[rr] += w1[rr, None] * eo[i1[rr], np.arange(T)]
        fused[rr] += w2[rr, None] * eo[i2[rr], np.arange(T)]
    return fused


MODE = "sparse"


def _spot_check(out, inputs, ntok=4, tol=5e-2):
    """Recompute a few tokens exactly on host; reject corrupted device runs."""
    try:
        idx = np.arange(0, N, N // ntok)[:ntok]
        c = np.asarray(inputs["combined"], np.float32)[idx]
        gh = _host_gelu(
            c @ np.asarray(inputs["gate_w1"], np.float32)
            + np.asarray(inputs["gate_b1"], np.float32)
        )
        lg = gh @ np.asarray(inputs["gate_w2"], np.float32) + np.asarray(
            inputs["gate_b2"], np.float32
        )
        p = np.exp(lg - lg.max(axis=1, keepdims=True))
        p /= p.sum(axis=1, keepdims=True)
        order = np.argsort(-p, axis=1, kind="stable")
        ew1 = np.asarray(inputs["ew1"], np.float32)
        eb1 = np.asarray(inputs["eb1"], np.float32)
        ew2 = np.asarray(inputs["ew2"], np.float32)
        eb2 = np.asarray(inputs["eb2"], np.float32)
        for t in range(ntok):
            i1, i2 = int(order[t, 0]), int(order[t, 1])
            w1 = p[t, i1] / (p[t, i1] + p[t, i2])
            exp_row = np.zeros(E, np.float32)
            for e, w in ((i1, w1), (i2, 1.0 - w1)):
                h = _host_gelu(c[t] @ ew1[e] + eb1[e])
                o = 1.0 / (1.0 + np.exp(-(h @ ew2[e] + eb2[e])))
                exp_row += w * o
            if not np.isfinite(out[idx[t]]).all():
                return False
            if np.abs(out[idx[t]] - exp_row).max() > tol:
                return False
        return True
    except Exception:
        return True  # never let the checker itself kill a good run


def kernel(**inputs):
    best = None
    if MODE == "sparse":
        for _ in range(3):  # transient device errors usually recover on retry
            try:
                out = kernel_sparse(**inputs)
            except Exception:
                continue
            if _spot_check(out, inputs):
                return out
            best = out
    try:
        out = kernel_dense(**inputs)
        if _spot_check(out, inputs) or best is None:
            return out
    except Exception:
        pass
    if best is not None:
        return best
    return kernel_dense(**inputs)


if __name__ == "__main__":  # dev smoke test only; harness imports kernel()
    import reference  # noqa: PLC0415 -- not needed when imported as a module

    inputs = {k: np.asarray(v) for k, v in reference.setup_inputs().items()}
    out = kernel(**inputs)
    print(out.shape, out.dtype)


# revision 31
# speedup vs baseline: 1.0157x; 1.0079x over previous
"""MoE fusion kernel for Trainium2 (8 NeuronCores, two-phase sparse routing).

Phase 1 (gate NEFF, all-bf16): computes gate logits on device.  bf16 logit
error is <= ~1e-2; any token whose 2nd-vs-3rd logit margin is below
MARGIN_THR is recomputed exactly (fp32) on the host before routing, so the
top-2 selection always matches the fp32 reference.  The combine weights
collapse to sigmoid(l1 - l2), which tolerates ~1e-2 logit error harmlessly.

Phase 2 (expert NEFF, bf16): host routes each token's top-2 experts into
24 uniform slots (3 per core, capacity S).  Each slot runs one expert's
Linear-GELU-Linear over its gathered tokens in a feature-major layout:

    x.T [1536, S] -> h.T = gelu(W1.T x.T) [3072, S] -> o.T = W2.T h.T [768, S]

Matmuls are weights-stationary with 512-column moving chunks (PSUM bank
limit); each matmul instruction costs ~N/2.4 + 43 ns, so big-N chunks and
low padding dominate the schedule.  Device returns sigmoid(o) in bf16; the
host applies the combine weights during scatter-add.
"""

import numpy as np

try:
    import concourse  # noqa: F401
except ImportError:  # pragma: no cover
    import sys

    sys.path.insert(0, "/opt/trn_rl_repo")

import concourse.bass as bass  # noqa: F401
import concourse.mybir as mybir
import concourse.tile as tile
from concourse import bacc
from concourse.bass_utils import run_bass_kernel_spmd

# Problem shapes (hardcoded per contest rules).
N, D, E, H, NE = 8192, 1536, 768, 3072, 12
NCORES = 8
T = N // NCORES  # 1024 tokens per core
P = 128
KO1 = D // P  # 12   k-tiles of the first expert matmul
FO1 = H // P  # 24   feature-tiles of h
KO2 = H // P  # 24   k-tiles of the second expert matmul
FO2 = E // P  # 6    feature-tiles of the output
GFO = E // P  # 6    feature-tiles of the gate hidden

F32 = mybir.dt.float32
BF16 = mybir.dt.bfloat16
AF = mybir.ActivationFunctionType
OP = mybir.AluOpType

GELU = AF.Gelu  # test.py sim-mode substitutes Tanh (CoreSim lacks Gelu)

EXP = 5  # expert slots per core (8*EXP slots total, assigned by load)
SUMS_MAX = 4096  # cap on per-core total slot capacity (SBUF) -> dense fallback
MARGIN_THR = 0.03  # host-recompute tokens with 2nd-3rd logit margin below this


def _chunks(total):
    """Column chunks <=512 (PSUM bank limit); even split for 512<S<=1024 so
    neither chunk drops under the ~100ns LDWEIGHTS issue floor."""
    if total <= 512:
        return [(0, total)]
    assert total <= 1024
    h = (total + 1) // 2
    return [(0, h), (h, total)]


# ======================================================================
# Phase-1 NEFF: bf16 gate -> logits.T [NE, T] per core
# ======================================================================


def build_nc_gate():
    nc = bacc.Bacc("TRN2", target_bir_lowering=False, debug=False, num_devices=NCORES)
    xTb = nc.dram_tensor("xTb", [P, KO1, T], BF16, kind="ExternalInput").ap()
    gw1 = nc.dram_tensor("gw1", [P, GFO, KO1, P], BF16, kind="ExternalInput").ap()
    gb1 = nc.dram_tensor("gb1", [P, GFO], F32, kind="ExternalInput").ap()
    gw2 = nc.dram_tensor("gw2", [P, GFO, NE], BF16, kind="ExternalInput").ap()
    lgT = nc.dram_tensor("lgT", [NE, T], F32, kind="ExternalOutput").ap()

    with tile.TileContext(nc) as tc:
        with (
            tc.tile_pool(name="sb", bufs=1) as sb,
            tc.tile_pool(name="lg", bufs=2) as lg,
            tc.tile_pool(name="ps", bufs=2, space="PSUM") as ps,
            tc.tile_pool(name="pl", bufs=2, space="PSUM") as pls,
        ):
            gw1_s = sb.tile([P, GFO, KO1, P], BF16)
            xTb_s = sb.tile([P, KO1, T], BF16)
            gb1_s = sb.tile([P, GFO], F32)
            gw2_s = sb.tile([P, GFO, NE], BF16)
            # interleave so that (gw1[fo0], xTb[t2=0]) land first; one strided
            # DMA per t2-chunk (piecewise issue serializes on the Sync engine)
            nc.sync.dma_start(gw1_s[:, 0], gw1[:, 0])
            nc.sync.dma_start(xTb_s[:, :, 0:512], xTb[:, :, 0:512])
            nc.sync.dma_start(gb1_s[:], gb1)
            nc.sync.dma_start(gw2_s[:], gw2)
            for fo in range(1, GFO):
                nc.sync.dma_start(gw1_s[:, fo], gw1[:, fo])
            for t2 in range(1, T // 512):
                nc.sync.dma_start(
                    xTb_s[:, :, t2 * 512 : (t2 + 1) * 512],
                    xTb[:, :, t2 * 512 : (t2 + 1) * 512],
                )
            ghT = sb.tile([P, GFO, T], BF16)

            for t2 in range(T // 512):
                sl = slice(t2 * 512, (t2 + 1) * 512)
                for fo in range(GFO):
                    pg = ps.tile([P, 512], F32, tag="pg")
                    for ko in range(KO1):
                        nc.tensor.matmul(
                            pg[:],
                            lhsT=gw1_s[:, fo, ko, :],
                            rhs=xTb_s[:, ko, sl],
                            start=(ko == 0),
                            stop=(ko == KO1 - 1),
                        )
                    nc.scalar.activation(
                        ghT[:, fo, sl], pg[:], GELU, bias=gb1_s[:, fo : fo + 1]
                    )
                pl = pls.tile([NE, 512], F32, tag="pl")
                for fo in range(GFO):
                    nc.tensor.matmul(
                        pl[:],
                        lhsT=gw2_s[:, fo, :],
                        rhs=ghT[:, fo, sl],
                        start=(fo == 0),
                        stop=(fo == GFO - 1),
                    )
                lt = lg.tile([NE, 512], F32, tag="lt")
                nc.vector.tensor_copy(lt[:], pl[:])
                nc.sync.dma_start(lgT[:, sl], lt[:])
    nc.compile()
    return nc


# ======================================================================
# Phase-2 NEFF: per-core EXP expert slots with per-slot capacities `sizes`
# (identical across cores; slot loads are classed by the host router).
# ======================================================================


def build_nc_exp(sizes):
    sizes = tuple(int(s) for s in sizes)
    Tc = sum(sizes)
    offs = [0]
    for s in sizes:
        offs.append(offs[-1] + s)
    nc = bacc.Bacc("TRN2", target_bir_lowering=False, debug=False, num_devices=NCORES)
    xTe = nc.dram_tensor("xTe", [P, KO1, Tc], BF16, kind="ExternalInput").ap()
    w1s = nc.dram_tensor("w1s", [EXP, FO1, P, KO1, P], BF16, kind="ExternalInput").ap()
    b1s = nc.dram_tensor("b1s", [P, EXP, FO1], F32, kind="ExternalInput").ap()
    w2s = nc.dram_tensor("w2s", [EXP, FO2, P, KO2, P], BF16, kind="ExternalInput").ap()
    b2s = nc.dram_tensor("b2s", [P, EXP, FO2], F32, kind="ExternalInput").ap()
    oT = nc.dram_tensor("oT", [P, FO2, Tc], BF16, kind="ExternalOutput").ap()

    import contextlib

    with tile.TileContext(nc) as tc, contextlib.ExitStack() as ctx:
        pers = ctx.enter_context(tc.tile_pool(name="pers", bufs=1))
        b1s_s = pers.tile([P, EXP, FO1], F32)
        b2s_s = pers.tile([P, EXP, FO2], F32)
        xTe_s = pers.tile([P, KO1, Tc], BF16)

        def fetch_xte(sl):
            # one strided DMA for the whole slot (DMA-issue on Sync costs
            # ~0.65us each; piecewise issue serializes the startup)
            nc.sync.dma_start(
                xTe_s[:, :, offs[sl] : offs[sl + 1]],
                xTe[:, :, offs[sl] : offs[sl + 1]],
            )

        fetch_xte(0)  # slot 0's tokens land first; later slots prefetch below
        nc.sync.dma_start(b1s_s[:], b1s)  # biases after: consumed only by ACT
        nc.sync.dma_start(b2s_s[:], b2s)

        w1pool = ctx.enter_context(tc.tile_pool(name="w1p", bufs=6))
        w2pool = ctx.enter_context(tc.tile_pool(name="w2p", bufs=4))
        hpool = ctx.enter_context(tc.tile_pool(name="hp", bufs=1))
        spool = ctx.enter_context(tc.tile_pool(name="sp", bufs=4))
        opool = ctx.enter_context(tc.tile_pool(name="op", bufs=4))
        psA = ctx.enter_context(tc.tile_pool(name="psA", bufs=4, space="PSUM"))
        psB = ctx.enter_context(tc.tile_pool(name="psB", bufs=4, space="PSUM"))

        for sl in range(EXP):
            S = sizes[sl]
            t0 = offs[sl]
            ch = _chunks(S)
            hbig = hpool.tile([P, KO2 * S], BF16, tag="ht")
            for fo in range(FO1):
                w1t = w1pool.tile([P, KO1, P], BF16, tag="w1t")
                nc.sync.dma_start(w1t[:], w1s[sl, fo])
                for a, b in ch:
                    pa = psA.tile([P, 512], F32, tag="psA")
                    for ko in range(KO1):
                        nc.tensor.matmul(
                            pa[:, : b - a],
                            lhsT=w1t[:, ko, :],
                            rhs=xTe_s[:, ko, t0 + a : t0 + b],
                            start=(ko == 0),
                            stop=(ko == KO1 - 1),
                        )
                    nc.scalar.activation(
                        hbig[:, fo * S + a : fo * S + b],
                        pa[:, : b - a],
                        GELU,
                        bias=b1s_s[:, sl, fo : fo + 1],
                    )
            if sl + 1 < EXP:
                fetch_xte(sl + 1)  # prefetch after this slot's w1 stream
            for fo2 in range(FO2):
                w2t = w2pool.tile([P, KO2, P], BF16, tag="w2t")
                nc.sync.dma_start(w2t[:], w2s[sl, fo2])
                for a, b in ch:
                    pb = psB.tile([P, 512], F32, tag="psB")
                    for ko in range(KO2):
                        nc.tensor.matmul(
                            pb[:, : b - a],
                            lhsT=w2t[:, ko, :],
                            rhs=hbig[:, ko * S + a : ko * S + b],
                            start=(ko == 0),
                            stop=(ko == KO2 - 1),
                        )
                    # device returns tanh(0.5*o + 0.5*b2) in bf16; the host
                    # applies sigmoid = 0.5 + 0.5*tanh during scatter-add
                    # (b2s is pre-halved)
                    ot = opool.tile([P, 512], BF16, tag="ot")
                    nc.scalar.activation(
                        ot[:, : b - a],
                        pb[:, : b - a],
                        AF.Tanh,
                        bias=b2s_s[:, sl, fo2 : fo2 + 1],
                        scale=0.5,
                    )
                    nc.sync.dma_start(oT[:, fo2, t0 + a : t0 + b], ot[:, : b - a])
    nc.compile()
    return nc


# ======================================================================
# Host side
# ======================================================================

_NC_CACHE = {}


def prep_shared(inputs):
    """Host-side relayout of the shared (replicated) tensors."""
    bf16 = mybir.dt.np(BF16)
    gate_w1 = np.asarray(inputs["gate_w1"], np.float32)
    gate_b1 = np.asarray(inputs["gate_b1"], np.float32)
    gate_w2 = np.asarray(inputs["gate_w2"], np.float32)
    ew1 = np.asarray(inputs["ew1"], np.float32)
    eb1 = np.asarray(inputs["eb1"], np.float32)
    ew2 = np.asarray(inputs["ew2"], np.float32)
    eb2 = np.asarray(inputs["eb2"], np.float32)

    return {
        # [P, GFO, KO1, P]: gw1[d, f] -> [p_k, fo, ko, p_f]
        "gw1": np.ascontiguousarray(
            gate_w1.reshape(KO1, P, GFO, P).transpose(1, 2, 0, 3)
        ).astype(bf16),
        "gb1": np.ascontiguousarray(gate_b1.reshape(GFO, P).T),
        "gw2": np.ascontiguousarray(
            gate_w2.reshape(GFO, P, NE).transpose(1, 0, 2)
        ).astype(bf16),
        "w1e": np.ascontiguousarray(
            ew1.reshape(NE, KO1, P, FO1, P).transpose(0, 3, 2, 1, 4)
        ).astype(bf16),
        "b1e": np.ascontiguousarray(eb1.reshape(NE, FO1, P).transpose(2, 0, 1)),
        "w2e": np.ascontiguousarray(
            ew2.reshape(NE, KO2, P, FO2, P).transpose(0, 3, 2, 1, 4)
        ).astype(bf16),
        "b2e": np.ascontiguousarray(
            (0.5 * eb2).reshape(NE, FO2, P).transpose(2, 0, 1)
        ),
    }


def prep_xTb(inputs):
    bf16 = mybir.dt.np(BF16)
    combined = np.asarray(inputs["combined"], np.float32)
    xTbs = []
    for c in range(NCORES):
        xt = np.ascontiguousarray(
            combined[c * T : (c + 1) * T].T.reshape(KO1, P, T).transpose(1, 0, 2)
        ).astype(bf16)
        xTbs.append(xt)
    return xTbs


def _host_gelu(x):
    try:
        from scipy.special import erf
    except ImportError:  # pragma: no cover
        import math

        _erf_u = np.frompyfunc(math.erf, 1, 1)

        def erf(v):
            return _erf_u(v).astype(v.dtype)

    return (0.5 * x * (1.0 + erf(x / np.sqrt(np.float32(2.0))))).astype(np.float32)


def fixup_logits(logits, inputs):
    """Recompute exact fp32 logits for tokens whose 2-vs-3 margin is unsafe."""
    srt = np.sort(logits, axis=1)
    margin = srt[:, -2] - srt[:, -3]
    unsafe = np.nonzero(margin < MARGIN_THR)[0]
    if len(unsafe) == 0:
        return logits
    c = np.asarray(inputs["combined"], np.float32)[unsafe]
    gh = _host_gelu(
        c @ np.asarray(inputs["gate_w1"], np.float32)
        + np.asarray(inputs["gate_b1"], np.float32)
    )
    lg = gh @ np.asarray(inputs["gate_w2"], np.float32) + np.asarray(
        inputs["gate_b2"], np.float32
    )
    logits = logits.copy()
    logits[unsafe] = lg
    return logits


def _mm_cost(S):
    """ns of PE issue time per (fo,ko) weight tile at slot capacity S:
    per chunk max(stream, ~100ns LDWEIGHTS floor)."""
    return sum(max((b - a) / 2.4 + 2.5, 100.0) for a, b in _chunks(S))


def _assign(sizes, cnt_desc):
    """Greedy bin-cover: experts (desc counts) onto NCORES bins per class.
    Returns per-class lists of (expert, amount) or None if infeasible."""
    avail = [NCORES] * len(sizes)
    cls_desc = sorted(range(len(sizes)), key=lambda j: -sizes[j])
    out = [[] for _ in sizes]
    for e, ce in cnt_desc:
        rem = ce
        while rem > 0:
            jbig = next((j for j in cls_desc if avail[j] > 0), None)
            if jbig is None:
                return None
            if rem > sizes[jbig]:
                avail[jbig] -= 1
                out[jbig].append((e, sizes[jbig]))
                rem -= sizes[jbig]
            else:
                cands = [j for j in cls_desc if avail[j] > 0 and sizes[j] >= rem]
                j = min(cands, key=lambda q: sizes[q]) if cands else jbig
                avail[j] -= 1
                out[j].append((e, rem))
                rem = 0
    return out


def _assign_exact(sizes, cnt_desc, budget=30000000):
    """Exact bin-cover (bounded DP): like _assign but complete.  Returns
    per-class lists of (expert, amount), or None if infeasible / over
    budget."""
    K = len(sizes)
    nodes = [budget]
    fail = set()

    def covers(need, avail):
        out = []

        def h(j, vec, cap):
            if nodes[0] <= 0:
                return
            nodes[0] -= 1
            if cap >= need:
                for q in range(len(vec)):
                    if vec[q] > 0 and cap - sizes[q] >= need:
                        return
                out.append(tuple(vec) + (0,) * (K - len(vec)))
                return
            if j == K:
                return
            for n in range(avail[j], -1, -1):
                h(j + 1, vec + [n], cap + n * sizes[j])

        h(0, [], 0)
        return out

    def go(ei, avail):
        if nodes[0] <= 0:
            return None
        if ei == len(cnt_desc):
            return []
        if (ei, avail) in fail:
            return None
        for vec in covers(cnt_desc[ei][1], avail):
            na = tuple(a - v for a, v in zip(avail, vec))
            if min(na) >= 0:
                rest = go(ei + 1, na)
                if rest is not None:
                    return [vec] + rest
        fail.add((ei, avail))
        return None

    vecs = go(0, (NCORES,) * K)
    if vecs is None or nodes[0] <= 0:
        return None
    out = [[] for _ in range(K)]
    for (e, ce), vec in zip(cnt_desc, vecs):
        rem = ce
        bins = [j for j in range(K) for _ in range(vec[j])]
        bins.sort(key=lambda j: -sizes[j])
        for j in bins:
            amt = min(rem, sizes[j])
            out[j].append((e, amt))
            rem -= amt
        if rem != 0:
            return None
    return out


def _best_sizes(cnt):
    """Search desc tuples of EXP multiples of 32 minimizing total PE issue
    cost subject to bin-cover feasibility."""
    cnt_desc = sorted(enumerate(cnt), key=lambda q: -q[1])
    best = [None]
    nodes = [0]

    def rec(prefix, remaining, maxv):
        if nodes[0] > 500000:
            return
        nodes[0] += 1
        if remaining == 0:
            if _assign(list(prefix), cnt_desc) is not None:
                cc = sum(_mm_cost(s) for s in prefix)
                if best[0] is None or cc < best[0][0]:
                    best[0] = (cc, tuple(prefix))
            return
        need = (sum(cnt) + NCORES - 1) // NCORES
        for v in range(min(maxv, 1024), 31, -32):
            if sum(prefix) + v * remaining < need:
                return
            pc = sum(_mm_cost(s) for s in prefix) + _mm_cost(v)
            if best[0] and pc + (remaining - 1) * 100.0 >= best[0][0]:
                continue
            rec(prefix + (v,), remaining - 1, v)

    rec(tuple(), EXP, 1024)
    if best[0] is None:  # fallback: uniform worst-case split
        S = max(32, int((max(cnt) + 31) // 32 * 32))
        return (min(S, 1024),) * EXP
    return best[0][1]


def route(logits):
    """Host softmax/top-2/normalize + slot assignment.

    NCORES*EXP slots in EXP size classes of NCORES bins each (core c runs
    one slot of every class, so all cores execute the same heterogeneous
    capacity tuple).  Class sizes are chosen by `_best_sizes` to minimize
    PE issue time subject to covering each expert's token count; the first
    slot is the smallest class so the initial xTe DMA lands fast.

    Returns (slots, sizes) where slots[c][j] = (expert, tokens, weights)
    for core c's j-th slot (len(tokens) <= sizes[j])."""
    lg = logits.astype(np.float32)
    m = lg.max(axis=1, keepdims=True)
    p = np.exp(lg - m)
    p /= p.sum(axis=1, keepdims=True)
    order = np.argsort(-p, axis=1, kind="stable")
    i1, i2 = order[:, 0], order[:, 1]
    r = np.arange(lg.shape[0])
    w1 = p[r, i1]
    w2 = p[r, i2]
    s = w1 + w2
    w1, w2 = w1 / s, w2 / s

    toks, wts = [], []
    for e in range(NE):
        t1 = np.nonzero(i1 == e)[0]
        t2 = np.nonzero(i2 == e)[0]
        toks.append(np.concatenate([t1, t2]))
        wts.append(np.concatenate([w1[t1], w2[t2]]).astype(np.float32))
    cnt = [len(t) for t in toks]
    cnt_desc = sorted(enumerate(cnt), key=lambda q: -q[1])

    cur = tuple(sorted(_best_sizes(cnt), reverse=True))
    percls = None
    # local descent: shrink one class by 32 while an exact bin-cover still
    # exists (the greedy cover used during the search is conservative)
    import time as _time

    deadline = _time.time() + 40.0
    improved = True
    while improved and _time.time() < deadline:
        improved = False
        for j in range(EXP):
            if _time.time() >= deadline:
                break
            cand = list(cur)
            cand[j] -= 32
            if cand[j] < 192:
                continue
            cs = tuple(sorted(cand, reverse=True))
            a = _assign_exact(cs, cnt_desc)
            if a is not None:
                cur, percls, improved = cs, a, True
                break
    if percls is None:
        percls = _assign_exact(cur, cnt_desc) or _assign(list(cur), cnt_desc)
    assert percls is not None
    # mid-size slot first (fast xTe(0) landing), largest second (runs while
    # later prefetches have slack), smallest last (cheap tail; its weight
    # stream no longer competes with xTe prefetches)
    perm = ([1, 0] + list(range(2, EXP))) if EXP > 1 else [0]
    sizes = tuple(cur[p] for p in perm)
    percls = [percls[p] for p in perm]

    consumed = [0] * NE
    # per class j, bins percls[j] (<= NCORES) distributed one per core
    grid = [[(0, np.zeros(0, np.int64), np.zeros(0, np.float32))] * EXP
            for _ in range(NCORES)]
    for j in range(EXP):
        for c, (e, amt) in enumerate(percls[j]):
            a = consumed[e]
            consumed[e] += amt
            grid[c][j] = (e, toks[e][a : a + amt], wts[e][a : a + amt])
    assert consumed == cnt
    return grid, sizes


def kernel_sparse(**inputs):
    bf16 = mybir.dt.np(BF16)
    shared = prep_shared(inputs)
    xTbs = prep_xTb(inputs)

    if "gate" not in _NC_CACHE:
        _NC_CACHE["gate"] = build_nc_gate()
    ncg = _NC_CACHE["gate"]
    gmaps = [
        {
            "xTb": xTbs[c],
            "gw1": shared["gw1"],
            "gb1": shared["gb1"],
            "gw2": shared["gw2"],
        }
        for c in range(NCORES)
    ]
    gres = run_bass_kernel_spmd(ncg, gmaps, core_ids=list(range(NCORES)))
    logits = np.concatenate(
        [gres.results[c]["lgT"].T for c in range(NCORES)]
    )  # [N, NE]
    logits = logits + np.asarray(inputs["gate_b2"], np.float32)
    logits = fixup_logits(logits, inputs)

    slots, sizes = route(logits)
    Tc = sum(sizes)
    if Tc > SUMS_MAX:  # extremely unbalanced routing: use the dense fallback
        return kernel_dense(**inputs)
    offs = [0]
    for s in sizes:
        offs.append(offs[-1] + s)

    if ("exp", sizes) not in _NC_CACHE:
        _NC_CACHE[("exp", sizes)] = build_nc_exp(sizes)
    nce = _NC_CACHE[("exp", sizes)]

    combined = np.asarray(inputs["combined"], np.float32)
    emaps = []
    for c in range(NCORES):
        eids = [slots[c][j][0] for j in range(EXP)]
        xg = np.zeros((Tc, D), np.float32)
        for j in range(EXP):
            tk = slots[c][j][1]
            xg[offs[j] : offs[j] + len(tk)] = combined[tk]
        emaps.append(
            {
                "xTe": np.ascontiguousarray(
                    xg.T.reshape(KO1, P, Tc).transpose(1, 0, 2)
                ).astype(bf16),
                "w1s": np.ascontiguousarray(shared["w1e"][eids]),
                "b1s": np.ascontiguousarray(shared["b1e"][:, eids, :]),
                "w2s": np.ascontiguousarray(shared["w2e"][eids]),
                "b2s": np.ascontiguousarray(shared["b2e"][:, eids, :]),
            }
        )
    _NC_CACHE["last_emaps"] = emaps
    eres = run_bass_kernel_spmd(nce, emaps, core_ids=list(range(NCORES)))

    fused = np.zeros((N, E), np.float32)
    for c in range(NCORES):
        # device returns tanh(o/2 + b2/2); sigmoid = 0.5 + 0.5*tanh
        rows = (
            eres.results[c]["oT"].astype(np.float32).transpose(2, 1, 0).reshape(Tc, E)
        )
        rows = 0.5 + 0.5 * rows
        for j in range(EXP):
            _, tk, wt = slots[c][j]
            np.add.at(
                fused,
                tk,
                wt[:, None] * rows[offs[j] : offs[j] + len(tk)],
            )
    return fused


# ======================================================================
# Dense fallback (every expert on every token; no routing dependence).
# Only used if routing is so unbalanced that S > S_MAX.
# ======================================================================


def build_nc_dense():
    nc = bacc.Bacc("TRN2", target_bir_lowering=False, debug=False, num_devices=NCORES)
    xTb = nc.dram_tensor("xTb", [P, KO1, T], BF16, kind="ExternalInput").ap()
    gw1 = nc.dram_tensor("gw1", [P, GFO, KO1, P], BF16, kind="ExternalInput").ap()
    gb1 = nc.dram_tensor("gb1", [P, GFO], F32, kind="ExternalInput").ap()
    gw2 = nc.dram_tensor("gw2", [P, GFO, NE], BF16, kind="ExternalInput").ap()
    w1e = nc.dram_tensor("w1e", [NE, FO1, P, KO1, P], BF16, kind="ExternalInput").ap()
    b1e = nc.dram_tensor("b1e", [P, NE, FO1], F32, kind="ExternalInput").ap()
    w2e = nc.dram_tensor("w2e", [NE, FO2, P, KO2, P], BF16, kind="ExternalInput").ap()
    b2e = nc.dram_tensor("b2e", [P, NE, FO2], F32, kind="ExternalInput").ap()
    lgT = nc.dram_tensor("lgT", [NE, T], F32, kind="ExternalOutput").ap()
    eoT = nc.dram_tensor("eoT", [NE, P, FO2, T], BF16, kind="ExternalOutput").ap()

    import contextlib

    with tile.TileContext(nc) as tc, contextlib.ExitStack() as ctx:
        pers = ctx.enter_context(tc.tile_pool(name="pers", bufs=1))
        xTb_s = pers.tile([P, KO1, T], BF16)
        nc.sync.dma_start(xTb_s[:], xTb)
        b1e_s = pers.tile([P, NE, FO1], F32)
        nc.sync.dma_start(b1e_s[:], b1e)
        b2e_s = pers.tile([P, NE, FO2], F32)
        nc.sync.dma_start(b2e_s[:], b2e)

        # gate
        with (
            tc.tile_pool(name="gsb", bufs=1) as gsb,
            tc.tile_pool(name="glg", bufs=2) as glg,
            tc.tile_pool(name="gps", bufs=2, space="PSUM") as gps,
            tc.tile_pool(name="gpl", bufs=2, space="PSUM") as gpl,
        ):
            gw1_s = gsb.tile([P, GFO, KO1, P], BF16)
            nc.sync.dma_start(gw1_s[:], gw1)
            gb1_s = gsb.tile([P, GFO], F32)
            nc.sync.dma_start(gb1_s[:], gb1)
            gw2_s = gsb.tile([P, GFO, NE], BF16)
            nc.sync.dma_start(gw2_s[:], gw2)
            ghT = gsb.tile([P, GFO, T], BF16)
            for t2 in range(T // 512):
                sl = slice(t2 * 512, (t2 + 1) * 512)
                for fo in range(GFO):
                    pg = gps.tile([P, 512], F32, tag="pg")
                    for ko in range(KO1):
                        nc.tensor.matmul(
                            pg[:],
                            lhsT=gw1_s[:, fo, ko, :],
                            rhs=xTb_s[:, ko, sl],
                            start=(ko == 0),
                            stop=(ko == KO1 - 1),
                        )
                    nc.scalar.activation(
                        ghT[:, fo, sl], pg[:], GELU, bias=gb1_s[:, fo : fo + 1]
                    )
                pl = gpl.tile([NE, 512], F32, tag="pl")
                for fo in range(GFO):
                    nc.tensor.matmul(
                        pl[:],
                        lhsT=gw2_s[:, fo, :],
                        rhs=ghT[:, fo, sl],
                        start=(fo == 0),
                        stop=(fo == GFO - 1),
                    )
                lt = glg.tile([NE, 512], F32, tag="lt")
                nc.vector.tensor_copy(lt[:], pl[:])
                nc.sync.dma_start(lgT[:, sl], lt[:])

        # experts (dense)
        w1pool = ctx.enter_context(tc.tile_pool(name="w1p", bufs=4))
        w2pool = ctx.enter_context(tc.tile_pool(name="w2p", bufs=3))
        hpool = ctx.enter_context(tc.tile_pool(name="hp", bufs=1))
        spool = ctx.enter_context(tc.tile_pool(name="sp", bufs=2))
        opool = ctx.enter_context(tc.tile_pool(name="op", bufs=2))
        psA = ctx.enter_context(tc.tile_pool(name="psA", bufs=3, space="PSUM"))
        psB = ctx.enter_context(tc.tile_pool(name="psB", bufs=3, space="PSUM"))
        ch = _chunks(T)
        for e in range(NE):
            hbig = hpool.tile([P, KO2 * T], BF16, tag="ht")
            for fo in range(FO1):
                w1t = w1pool.tile([P, KO1, P], BF16, tag="w1t")
                nc.sync.dma_start(w1t[:], w1e[e, fo])
                for a, b in ch:
                    pa = psA.tile([P, 512], F32, tag="psA")
                    for ko in range(KO1):
                        nc.tensor.matmul(
                            pa[:, : b - a],
                            lhsT=w1t[:, ko, :],
                            rhs=xTb_s[:, ko, a:b],
                            start=(ko == 0),
                            stop=(ko == KO1 - 1),
                        )
                    nc.scalar.activation(
                        hbig[:, fo * T + a : fo * T + b],
                        pa[:, : b - a],
                        GELU,
                        bias=b1e_s[:, e, fo : fo + 1],
                    )
            for fo2 in range(FO2):
                w2t = w2pool.tile([P, KO2, P], BF16, tag="w2t")
                nc.sync.dma_start(w2t[:], w2e[e, fo2])
                for a, b in ch:
                    pb = psB.tile([P, 512], F32, tag="psB")
                    for ko in range(KO2):
                        nc.tensor.matmul(
                            pb[:, : b - a],
                            lhsT=w2t[:, ko, :],
                            rhs=hbig[:, ko * T + a : ko * T + b],
                            start=(ko == 0),
                            stop=(ko == KO2 - 1),
                        )
                    st = spool.tile([P, 512], F32, tag="st")
                    nc.scalar.activation(
                        st[:, : b - a],
                        pb[:, : b - a],
                        AF.Tanh,
                        bias=b2e_s[:, e, fo2 : fo2 + 1],
                        scale=0.5,
                    )
                    ot = opool.tile([P, 512], BF16, tag="ot")
                    nc.vector.tensor_scalar(
                        ot[:, : b - a], st[:, : b - a], 0.5, 0.5, OP.mult, OP.add
                    )
                    nc.sync.dma_start(eoT[e, :, fo2, a:b], ot[:, : b - a])
    nc.compile()
    return nc


def kernel_dense(**inputs):
    if "dense" not in _NC_CACHE:
        _NC_CACHE["dense"] = build_nc_dense()
    nc = _NC_CACHE["dense"]
    shared = prep_shared(inputs)
    xTbs = prep_xTb(inputs)
    in_maps = [
        {
            "xTb": xTbs[c],
            "gw1": shared["gw1"],
            "gb1": shared["gb1"],
            "gw2": shared["gw2"],
            "w1e": shared["w1e"],
            "b1e": shared["b1e"],
            "w2e": shared["w2e"],
            "b2e": shared["b2e"],
        }
        for c in range(NCORES)
    ]
    res = run_bass_kernel_spmd(nc, in_maps, core_ids=list(range(NCORES)))

    logits = np.concatenate([res.results[c]["lgT"].T for c in range(NCORES)])
    logits = logits + np.asarray(inputs["gate_b2"], np.float32)
    logits = fixup_logits(logits, inputs)
    lg = logits
    m = lg.max(axis=1, keepdims=True)
    p = np.exp(lg - m)
    p /= p.sum(axis=1, keepdims=True)
    order = np.argsort(-p, axis=1, kind="stable")
    i1, i2 = order[:, 0], order[:, 1]
    r = np.arange(lg.shape[0])
    w1 = p[r, i1]
    w2 = p[r, i2]
    s = w1 + w2
    w1, w2 = w1 / s, w2 / s

    fused = np.zeros((N, E), np.float32)
    for c in range(NCORES):
        eo = res.results[c]["eoT"].astype(np.float32)  # [NE, P, FO2, T]
        eo = eo.transpose(0, 3, 2, 1).reshape(NE, T, E)  # [NE, T, E]
        rr = np.arange(c * T, (c + 1) * T)
        fused[rr] += w1[rr, None] * eo[i1[rr], np.arange(T)]
        fused[rr] += w2[rr, None] * eo[i2[rr], np.arange(T)]
    return fused


MODE = "sparse"


def _spot_check(out, inputs, ntok=4, tol=5e-2):
    """Recompute a few tokens exactly on host; reject corrupted device runs."""
    try:
        idx = np.arange(0, N, N // ntok)[:ntok]
        c = np.asarray(inputs["combined"], np.float32)[idx]
        gh = _host_gelu(
            c @ np.asarray(inputs["gate_w1"], np.float32)
            + np.asarray(inputs["gate_b1"], np.float32)
        )
        lg = gh @ np.asarray(inputs["gate_w2"], np.float32) + np.asarray(
            inputs["gate_b2"], np.float32
        )
        p = np.exp(lg - lg.max(axis=1, keepdims=True))
        p /= p.sum(axis=1, keepdims=True)
        order = np.argsort(-p, axis=1, kind="stable")
        ew1 = np.asarray(inputs["ew1"], np.float32)
        eb1 = np.asarray(inputs["eb1"], np.float32)
        ew2 = np.asarray(inputs["ew2"], np.float32)
        eb2 = np.asarray(inputs["eb2"], np.float32)
        for t in range(ntok):
            i1, i2 = int(order[t, 0]), int(order[t, 1])
            w1 = p[t, i1] / (p[t, i1] + p[t, i2])
            exp_row = np.zeros(E, np.float32)
            for e, w in ((i1, w1), (i2, 1.0 - w1)):
                h = _host_gelu(c[t] @ ew1[e] + eb1[e])
                o = 1.0 / (1.0 + np.exp(-(h @ ew2[e] + eb2[e])))
                exp_row += w * o
            if not np.isfinite(out[idx[t]]).all():
                return False
            if np.abs(out[idx[t]] - exp_row).max() > tol:
                return False
        return True
    except Exception:
        return True  # never let the checker itself kill a good run


def kernel(**inputs):
    best = None
    if MODE == "sparse":
        for _ in range(3):  # transient device errors usually recover on retry
            try:
                out = kernel_sparse(**inputs)
            except Exception:
                continue
            if _spot_check(out, inputs):
                return out
            best = out
    try:
        out = kernel_dense(**inputs)
        if _spot_check(out, inputs) or best is None:
            return out
    except Exception:
        pass
    if best is not None:
        return best
    return kernel_dense(**inputs)


if __name__ == "__main__":  # dev smoke test only; harness imports kernel()
    import reference  # noqa: PLC0415 -- not needed when imported as a module

    inputs = {k: np.asarray(v) for k, v in reference.setup_inputs().items()}
    out = kernel(**inputs)
    print(out.shape, out.dtype)
